# revision 6
# baseline (speedup 1.0000x reference)
"""MHA (RoPE + causal softmax attention + out-proj) on 8 NeuronCores — v3.

Sharding: DP4 x TP2 (core c: batch c % 4, head-group c // 4; 8 heads/core).
Host sums the two head-group partial outputs per batch and transposes.

Key structure (tuned against the TimelineSim cost model):
  * Phase A (QKV) matmuls run in fp8 e4m3 DoubleRow with a hi/lo split of
    both operands (x ~ xh + xl/16, 64w ~ wh + wl/16, lo*lo dropped):
    3 DR instructions per d-tile pair = 0.75x the bf16 instruction cost at
    better-than-bf16 accuracy. PSUM carries 1024*qkv; the 2^-10 unscale is
    folded into the RoPE-cast / v copy scales.
  * q/k comps are host-permuted to (evens | odds) within each head so RoPE
    reads contiguous PSUM blocks: 2 ACT casts + 6 DVE bf16 ops (4x mode).
  * Softmax: exp(alpha*s - 8ln2) -> fp16 exp tiles; denominator accumulated
    with DVE adds + ONE ones-matmul per (head, qc) instead of a ones-matmul
    per k-tile. Causal diagonal tiles are column-trimmed; a single [128,128]
    triangle mask remains.
  * v never round-trips DRAM (PSUM -> SBUF fp16 copy, resident).
  * Emission interleaves head-group-1 QKV tiles into the attention loop of
    head-group-0 so the exp-bound stretch of attention overlaps the
    PE-bound QKV GEMM instead of stalling the tensor engine.
"""

import numpy as np
import ml_dtypes

import concourse.bass as bass
import concourse.tile as tile
import concourse.mybir as mybir
from concourse import bacc
from concourse.bass_utils import run_bass_kernel_spmd

BF16 = ml_dtypes.bfloat16
F8NP = ml_dtypes.float8_e4m3
F32 = mybir.dt.float32
BF = mybir.dt.bfloat16
F16 = mybir.dt.float16
F8 = mybir.dt.float8e4
DR = mybir.MatmulPerfMode.DoubleRow

B, L, D, H, HD = 4, 2048, 2048, 16, 128
NH = 8                      # heads per core
DL = NH * HD                # 1024 local head dims
ROPE_BASE = 10000.0
ALPHA = float(HD) ** -0.5
EXP_BIAS = -8.0 * float(np.log(2.0))   # exp(a*s - 8ln2): keeps fp16 sums safe

LT = L // 128               # 16 L-tiles
DT = D // 128               # 16 D(contract)-tiles
NCH = 6                     # qkv chunks of 512 comps: q03,k03,v03,q47,k47,v47
QC = L // 512               # 4 q-chunks of 512
KT = L // 128               # 16 k-tiles


def _chunk_kind(c):
    # chunk order: q(heads0-3), k(0-3), v(0-3), q(4-7), k(4-7), v(4-7)
    return ("q", "k", "v")[c % 3], c // 3


def build_program(phases="ABC", la=3, scb=3, ypb=1, psab=3, patb=1, paob=3,
                  pbab=4, take=4, takes=None):
    nc = bacc.Bacc("TRN2", target_bir_lowering=False, debug=False, num_devices=8)

    # x hi/lo fp8 planes: hi = fp8(x), lo = fp8(16*(x-hi))
    xThi = nc.dram_tensor("xThi", [D, L], F8, kind="ExternalInput").ap()
    xTlo = nc.dram_tensor("xTlo", [D, L], F8, kind="ExternalInput").ap()
    # w planes: wA = fp8(16*w64_hi) [D, 3DL]; wB [D, 2*3DL]: per 512-chunk,
    # 1024 cols = (w64_hi 512 | w16_lo 512)
    wA = nc.dram_tensor("wA", [D, 3 * DL], F8, kind="ExternalInput").ap()
    wB = nc.dram_tensor("wB", [D, 6 * DL], F8, kind="ExternalInput").ap()
    woA = nc.dram_tensor("woA", [DL, L], F8, kind="ExternalInput").ap()
    woB = nc.dram_tensor("woB", [DL, 2 * L], F8, kind="ExternalInput").ap()
    chalf = nc.dram_tensor("chalf", [L, 256], BF, kind="ExternalInput").ap()
    shalf = nc.dram_tensor("shalf", [L, 256], BF, kind="ExternalInput").ap()
    tri = nc.dram_tensor("tri", [128, 128], F16, kind="ExternalInput").ap()
    outT = nc.dram_tensor("outT", [D, L], BF, kind="ExternalOutput").ap()

    # DRAM staging for rotated q/k, split per head-group so group-0 attention
    # does not depend on group-1 writes
    qkrot = [[nc.dram_tensor(f"{nm}rot{g}", [L, 512], BF, kind="Internal").ap()
              for g in range(2)] for nm in ("q", "k")]

    doA = "A" in phases
    doB = "B" in phases
    doC = "C" in phases

    with tile.TileContext(nc) as tc:
        outer_cm = tc.tile_pool(name="outer", bufs=1)
        pb_cm = tc.tile_pool(name="pBqk", bufs=2, side="right")
        pbm_cm = tc.tile_pool(name="pBm", bufs=1, side="right")
        pby03_cm = tc.tile_pool(name="pBy03", bufs=1)
        pba_cm = tc.tile_pool(name="pBa", bufs=pbab)
        pbr_cm = tc.tile_pool(name="pBr", bufs=1)
        pbv_cm = tc.tile_pool(name="pBv", bufs=1)
        pbd_cm = tc.tile_pool(name="pBd", bufs=2)
        P = {}
        with outer_cm as outer, pb_cm as pb, pbm_cm as pbm, \
             pby03_cm as pby03, pba_cm as pba, pbr_cm as pbr, \
             pbv_cm as pbv, pbd_cm as pbd:
            # v for both head groups, resident SBUF: [128(kpos), LT, 512]
            vsb = [outer.tile([128, LT, 512], F16, name=f"vsb{g}",
                              tag=f"vsb{g}") for g in range(2)]
            ones128 = outer.tile([128, 128], F16, name="ones128", tag="oc")
            nc.vector.memset(ones128, 1.0)
            ebias = outer.tile([128, 1], F32, name="ebias", tag="ebias")
            nc.vector.memset(ebias, EXP_BIAS)
            trit = pbm.tile([128, 128], F16, name="tri", tag="tri")
            nc.sync.dma_start(out=trit, in_=tri)

            qts = {}
            yall = {}
            yall[0] = pby03.tile([128, 4, 2, L], F8, name="yall0", tag="yall0")
            if not doB:
                nc.vector.memset(yall[0], 0.0)

            # ---------------- phase B helpers ----------------
            def load_qk(h):
                grp, hh = h // 4, h % 4
                qt = pb.tile([128, L], BF, name="qt", tag="qt")
                kt = pb.tile([128, L], BF, name="kt", tag="kt")
                for t, src in ((qt, qkrot[0][grp]), (kt, qkrot[1][grp])):
                    nc.sync.dma_start_transpose(
                        out=t, in_=src[:, hh * 128:(hh + 1) * 128])
                return qt, kt

            def emit_B_qc(h, qt, kt, qc):
                grp, hh = h // 4, h % 4
                nkt = 4 * qc + 4
                ypsum = P["psy"].tile([128, 512], F32, name="ypsum", tag="yp")
                dacc = pbd.tile([128, 512], F16, name="dacc", tag="dacc")
                ats = {}

                def emit_score(j):
                    m = j - 4 * qc
                    off = 128 * m if m > 0 else 0
                    w = 512 - off
                    sc = P["pss"].tile([128, 512], F32, name="sc", tag="sc")
                    nc.tensor.matmul(
                        sc[:, 0:w], kt[:, j * 128:(j + 1) * 128],
                        qt[:, qc * 512 + off:(qc + 1) * 512],
                        start=True, stop=True)
                    at = pba.tile([128, 512], F16, name="at", tag="at")
                    nc.scalar.activation(
                        out=at[:, 0:w], in_=sc[:, 0:w],
                        func=mybir.ActivationFunctionType.Exp,
                        scale=ALPHA, bias=ebias)
                    if m >= 0:
                        nc.vector.tensor_mul(at[:, 0:128], at[:, 0:128], trit)
                    ats[j] = (at, off, w)

                for j in range(min(la, nkt)):
                    emit_score(j)
                prev = None
                for j in range(nkt):
                    if j + la < nkt:
                        emit_score(j + la)
                    at, off, w = ats.pop(j)
                    nc.tensor.matmul(
                        ypsum[:, off:512],
                        vsb[grp][:, j, hh * 128:(hh + 1) * 128],
                        at[:, 0:w],
                        start=(j == 0), stop=(j == nkt - 1),
                        skip_group_check=True)
                    if j == 0:
                        prev = at
                    elif j == 1:
                        if qc == 0:
                            nc.vector.tensor_copy(
                                out=dacc[:, 0:128], in_=prev[:, 0:128])
                            nc.vector.tensor_add(
                                dacc[:, 128:512], prev[:, 128:512],
                                at[:, 0:w])
                        else:
                            nc.vector.tensor_add(dacc, prev, at)
                    else:
                        nc.vector.tensor_add(
                            dacc[:, off:512], dacc[:, off:512], at[:, 0:w])
                dpsum = P["psd"].tile([128, 512], F32, name="dpsum", tag="dp")
                nc.tensor.matmul(dpsum, ones128, dacc, start=True, stop=True)
                rbs = pbr.tile([128, 512], BF, name="rbs", tag="rbs")
                with nc.allow_low_precision("softmax recip bf16"):
                    nc.vector.reciprocal(out=rbs, in_=dpsum)
                qsl = slice(qc * 512, (qc + 1) * 512)
                ya = yall[grp]
                yb = pbv.tile([128, 512], BF, name="yb", tag="yb")
                nc.vector.tensor_mul(yb, ypsum, rbs)
                nc.gpsimd.tensor_copy(out=ya[:, hh, 1, qsl], in_=yb)
                yd = pbv.tile([128, 512], BF, name="yd", tag="yd")
                nc.gpsimd.tensor_sub(yd, yb, ya[:, hh, 1, qsl])
                nc.gpsimd.tensor_scalar_mul(ya[:, hh, 0, qsl], yd, 16.0)

            # ---------------- phase A scope + interleave ----------------
            with tc.tile_pool(name="pA", bufs=1) as pa, \
                 tc.tile_pool(name="pAw", bufs=2) as paw, \
                 tc.tile_pool(name="pAt", bufs=patb) as pat, \
                 tc.tile_pool(name="pAo", bufs=paob) as pao:
                xall = pa.tile([128, DT, 2, L], F8, name="xall", tag="xall")
                c_sb = pa.tile([128, LT, 256], BF, name="c_sb", tag="c_sb")
                s_sb = pa.tile([128, LT, 256], BF, name="s_sb", tag="s_sb")
                wch = {}

                def load_wch(c):
                    if c >= NCH or c in wch:
                        return
                    wa = paw.tile([128, DT, 512], F8, name="wchA", tag="wchA")
                    wb = paw.tile([128, DT, 2, 512], F8, name="wchB",
                                  tag="wchB")
                    wAr = wA[:, c * 512:(c + 1) * 512].rearrange(
                        "(d p) e -> p d e", p=128)
                    wBr = wB[:, c * 1024:(c + 1) * 1024].rearrange(
                        "(d p) e -> p d e", p=128)
                    wbf = wb.rearrange("p d t e -> p d (t e)")
                    for d4 in range(DT // 4):
                        sl = slice(4 * d4, 4 * d4 + 4)
                        nc.sync.dma_start(out=wa[:, sl, :], in_=wAr[:, sl, :])
                        nc.sync.dma_start(out=wbf[:, sl, :], in_=wBr[:, sl, :])
                    wch[c] = (wa, wb)

                def load_x():
                    wa = paw.tile([128, DT, 512], F8, name="wchA", tag="wchA")
                    wb = paw.tile([128, DT, 2, 512], F8, name="wchB",
                                  tag="wchB")
                    wAr = wA[:, 0:512].rearrange("(d p) e -> p d e", p=128)
                    wBr = wB[:, 0:1024].rearrange("(d p) e -> p d e", p=128)
                    wbf = wb.rearrange("p d t e -> p d (t e)")
                    xhr = xThi.rearrange("(d p) l -> p d l", p=128)
                    xlr = xTlo.rearrange("(d p) l -> p d l", p=128)
                    for d2 in range(DT // 2):
                        sl = slice(2 * d2, 2 * d2 + 2)
                        nc.sync.dma_start(out=xall[:, sl, 1, :],
                                          in_=xhr[:, sl, :])
                        nc.sync.dma_start(out=xall[:, sl, 0, :],
                                          in_=xlr[:, sl, :])
                        if d2 % 2 == 1:
                            sl4 = slice(2 * d2 - 2, 2 * d2 + 2)
                            nc.sync.dma_start(out=wa[:, sl4, :],
                                              in_=wAr[:, sl4, :])
                            nc.sync.dma_start(out=wbf[:, sl4, :],
                                              in_=wBr[:, sl4, :])
                            i2 = slice(d2 - 1, d2 + 1)
                            for t_sb, t_dr in ((c_sb, chalf), (s_sb, shalf)):
                                nc.sync.dma_start(
                                    out=t_sb[:, i2, :],
                                    in_=t_dr.rearrange("(i p) g -> p i g",
                                                       p=128)[:, i2, :])
                    # rope tables for the second half of the l range trail
                    for t_sb, t_dr in ((c_sb, chalf), (s_sb, shalf)):
                        nc.sync.dma_start(
                            out=t_sb[:, 8:LT, :],
                            in_=t_dr.rearrange("(i p) g -> p i g",
                                               p=128)[:, 8:LT, :])
                    wch[0] = (wa, wb)

                def emit_A_alpha(c, i):
                    wa, _ = wch[c]
                    ls = slice(i * 128, (i + 1) * 128)
                    pnat = P["psa"].tile([128, 512], F32, name="pnat",
                                         tag="pnat")
                    for d2 in range(DT // 2):
                        nc.tensor.matmul(
                            pnat,
                            xall[:, 2 * d2:2 * d2 + 2, 1, ls],
                            wa[:, 2 * d2:2 * d2 + 2, :],
                            start=(d2 == 0), stop=False, perf_mode=DR)
                    return pnat

                def emit_A_finish(c, i, pnat):
                    _, wb = wch[c]
                    if i == 8:
                        load_wch(c + 1)
                    ls = slice(i * 128, (i + 1) * 128)
                    for d in range(DT):
                        nc.tensor.matmul(
                            pnat,
                            xall[:, d, :, ls],
                            wb[:, d, :, :],
                            start=False, stop=(d == DT - 1), perf_mode=DR)
                    emit_A_rope(c, i, pnat)

                def emit_A_rope(c, i, pnat):
                    kind, grp = _chunk_kind(c)
                    ls = slice(i * 128, (i + 1) * 128)
                    if kind == "v":
                        nc.scalar.activation(
                            out=vsb[grp][:, i, :], in_=pnat,
                            func=mybir.ActivationFunctionType.Copy,
                            scale=1.0 / 1024.0)
                        return
                    # RoPE: per-head comps are permuted (evens | odds)
                    pv = pnat.rearrange("p (hh t z) -> p hh t z",
                                        hh=4, t=2, z=64)
                    x1 = pat.tile([128, 256], BF, name="x1", tag="x1")
                    nc.scalar.activation(
                        out=x1, in_=pv[:, :, 0, :],
                        func=mybir.ActivationFunctionType.Copy,
                        scale=1.0 / 1024.0)
                    x2 = pat.tile([128, 256], BF, name="x2", tag="x2")
                    nc.scalar.activation(
                        out=x2, in_=pv[:, :, 1, :],
                        func=mybir.ActivationFunctionType.Copy,
                        scale=1.0 / 1024.0)
                    ct = c_sb[:, i, :]
                    st = s_sb[:, i, :]
                    t1 = pat.tile([128, 256], BF, name="t1", tag="t1")
                    nc.vector.tensor_mul(t1, x1, ct)
                    t2 = pat.tile([128, 256], BF, name="t2", tag="t2")
                    nc.vector.tensor_mul(t2, x2, st)
                    t3 = pat.tile([128, 256], BF, name="t3", tag="t3")
                    nc.vector.tensor_mul(t3, x2, ct)
                    t4 = pat.tile([128, 256], BF, name="t4", tag="t4")
                    nc.vector.tensor_mul(t4, x1, st)
                    ro = pao.tile([128, 512], BF, name="ro", tag="ro")
                    rv = ro.rearrange("p (hh t z) -> p hh t z", hh=4, t=2, z=64)
                    nc.vector.tensor_sub(rv[:, :, 0, :], t1, t2)
                    nc.vector.tensor_add(rv[:, :, 1, :], t3, t4)
                    dst = qkrot[0 if kind == "q" else 1][grp]
                    nc.sync.dma_start(out=dst[ls, :], in_=ro)

                def emit_A_tile(c, i):
                    emit_A_finish(c, i, emit_A_alpha(c, i))

                def emit_A_dmajor(c, tiles):
                    # d-major across several open psum groups: every arriving
                    # x/w piece-group unlocks one alpha+2 betas per open tile
                    wa, wb = wch[c]
                    pns = [P["psa"].tile([128, 512], F32, name="pnat",
                                         tag="pnat") for _ in tiles]
                    for d2 in range(DT // 2):
                        for t, i in enumerate(tiles):
                            ls = slice(i * 128, (i + 1) * 128)
                            nc.tensor.matmul(
                                pns[t],
                                xall[:, 2 * d2:2 * d2 + 2, 1, ls],
                                wa[:, 2 * d2:2 * d2 + 2, :],
                                start=(d2 == 0), stop=False, perf_mode=DR)
                        for dd in (2 * d2, 2 * d2 + 1):
                            for t, i in enumerate(tiles):
                                ls = slice(i * 128, (i + 1) * 128)
                                nc.tensor.matmul(
                                    pns[t],
                                    xall[:, dd, :, ls],
                                    wb[:, dd, :, :],
                                    start=False, stop=(dd == DT - 1),
                                    perf_mode=DR)
                    for t, i in enumerate(tiles):
                        emit_A_rope(c, i, pns[t])

                # ---- emission: A(g0), then heads 0-3 x A(g1) ----
                if doA:
                    load_x()
                    # chunk 0 lead-in: 7 psum banks, d-major
                    with tc.tile_pool(name="psA0", bufs=8,
                                      space="PSUM") as psa0:
                        P["psa"] = psa0
                        emit_A_dmajor(0, range(8))
                        for i in range(8, LT):
                            emit_A_tile(0, i)
                with tc.tile_pool(name="psS", bufs=scb, space="PSUM") as pss, \
                     tc.tile_pool(name="psY", bufs=ypb, space="PSUM") as psy, \
                     tc.tile_pool(name="psD", bufs=1, space="PSUM") as psd:
                    P["pss"], P["psy"], P["psd"] = pss, psy, psd
                    with tc.tile_pool(name="psA", bufs=psab,
                                      space="PSUM") as psa:
                        P["psa"] = psa
                        if doA:
                            for c in (1, 2):
                                for i in range(LT):
                                    emit_A_tile(c, i)
                        g1 = [(c, i) for c in (3, 4, 5) for i in range(LT)] \
                            if doA else []
                        gi = 0
                        qts.clear()
                        slot = 0
                        for h in range(3 if doB else 0):
                            if h not in qts:
                                qts[h] = load_qk(h)
                            qt, kt = qts.pop(h)
                            for qc in range(QC):
                                emit_B_qc(h, qt, kt, qc)
                                if qc == 0 and h < 3 and doB:
                                    qts[h + 1] = load_qk(h + 1)
                                tk = takes[slot] if takes else take
                                slot += 1
                                for _ in range(tk):
                                    if gi < len(g1):
                                        emit_A_tile(*g1[gi])
                                        gi += 1
                        while gi < len(g1):
                            emit_A_tile(*g1[gi])
                            gi += 1

            # ---------------- heads 4-7 (qc-major) + phase C ----------
            with tc.tile_pool(name="pBy47", bufs=1) as pby47, \
                 tc.tile_pool(name="pCo", bufs=4) as pco, \
                 tc.tile_pool(name="psS3", bufs=scb, space="PSUM") as pss3, \
                 tc.tile_pool(name="psY3", bufs=ypb, space="PSUM") as psy3, \
                 tc.tile_pool(name="psD3", bufs=1, space="PSUM") as psd3, \
                 tc.tile_pool(name="psC", bufs=2, space="PSUM") as psc:
                P["pss"], P["psy"], P["psd"] = pss3, psy3, psd3
                qk47 = dict(qts)   # h3 was prefetched into the pb pool
                for h in range(4, NH if doB else 4):
                    grp, hh = h // 4, h % 4
                    qt = pby47.tile([128, L], BF, name=f"qt{h}", tag=f"qt{h}")
                    kt = pby47.tile([128, L], BF, name=f"kt{h}", tag=f"kt{h}")
                    for t, src in ((qt, qkrot[0][grp]), (kt, qkrot[1][grp])):
                        nc.sync.dma_start_transpose(
                            out=t, in_=src[:, hh * 128:(hh + 1) * 128])
                    qk47[h] = (qt, kt)
                yall[1] = pby47.tile([128, 4, 2, L], F8, name="yall1",
                                     tag="yall1")
                if not doB:
                    nc.vector.memset(yall[1], 0.0)
                woat = pby47.tile([128, NH, L], F8, name="woat", tag="woat")
                nc.sync.dma_start(
                    out=woat, in_=woA.rearrange("(dd p) e -> p dd e", p=128))
                wobt = pby47.tile([128, NH, 2, L], F8, name="wobt", tag="wobt")
                nc.sync.dma_start(
                    out=wobt.rearrange("p dd t e -> p dd (t e)"),
                    in_=woB.rearrange("(dd p) e -> p dd e", p=128))

                def emit_C(e, qc):
                    esl = slice(e * 128, (e + 1) * 128)
                    qsl = slice(qc * 512, (qc + 1) * 512)
                    op = psc.tile([128, 512], F32, name="op", tag="op")
                    for g in range(2):
                        for p2 in range(2):
                            dd = 4 * g + 2 * p2
                            nc.tensor.matmul(
                                op, woat[:, dd:dd + 2, esl],
                                yall[g][:, 2 * p2:2 * p2 + 2, 1, qsl],
                                start=(g == 0 and p2 == 0), stop=False,
                                perf_mode=DR)
                    for g in range(2):
                        for hh in range(4):
                            nc.tensor.matmul(
                                op, wobt[:, 4 * g + hh, :, esl],
                                yall[g][:, hh, :, qsl],
                                start=False, stop=(g == 1 and hh == 3),
                                perf_mode=DR)
                    ot = pco.tile([128, 512], BF, name="ot", tag="ot")
                    if (e + qc) % 2 == 0:
                        nc.scalar.activation(
                            out=ot, in_=op,
                            func=mybir.ActivationFunctionType.Copy,
                            scale=1.0 / 1024.0)
                    else:
                        nc.vector.tensor_scalar_mul(ot, op, 1.0 / 1024.0)
                    nc.sync.dma_start(
                        out=outT[e * 128:(e + 1) * 128,
                                 qc * 512:(qc + 1) * 512],
                        in_=ot)

                for qc in range(QC if doB else 0):
                    for h in range(3, NH):
                        emit_B_qc(h, qk47[h][0], qk47[h][1], qc)
                    if doC and qc > 0:
                        for e in range(DT):
                            emit_C(e, qc - 1)
                if doC:
                    for qc in ([3] if doB else range(QC)):
                        for e in range(DT):
                            emit_C(e, qc)
    nc.compile()
    return nc


_NC_CACHE = None


def _get_program():
    global _NC_CACHE
    if _NC_CACHE is None:
        _NC_CACHE = build_program()
    return _NC_CACHE


def _f8(a):
    return np.clip(np.asarray(a, np.float64), -240.0, 240.0).astype(F8NP)


# within each head's 128 comps: evens first, then odds
_PERM512 = np.concatenate(
    [np.concatenate([np.arange(h * 128, (h + 1) * 128, 2),
                     np.arange(h * 128 + 1, (h + 1) * 128, 2)])
     for h in range(4)])


def _host_inputs(x, w_qkv, w_o):
    inv = 1.0 / (ROPE_BASE ** (np.arange(0, HD, 2, dtype=np.float64) / HD))
    ang = np.arange(L, dtype=np.float64)[:, None] * inv[None, :]
    chalf = np.tile(np.cos(ang), (1, 4)).astype(BF16)          # [L, 256]
    shalf = np.tile(np.sin(ang), (1, 4)).astype(BF16)
    p = np.arange(128)[:, None]
    f = np.arange(128)[None, :]
    tri = (p <= f).astype(np.float16)                          # [128, 128]

    xhi_all, xlo_all = [], []
    for b in range(B):
        xb = np.asarray(x[b], np.float64)                      # [L, D]
        xh = _f8(xb)
        xl = _f8(16.0 * (xb - xh.astype(np.float64)))
        xhi_all.append(np.ascontiguousarray(xh.T))
        xlo_all.append(np.ascontiguousarray(xl.T))

    in_maps = []
    for c in range(8):
        b, g = c % 4, c // 4
        qr = w_qkv[g * DL:(g + 1) * DL]
        kr = w_qkv[D + g * DL:D + (g + 1) * DL]
        vr = w_qkv[2 * D + g * DL:2 * D + (g + 1) * DL]
        qp = qr[np.concatenate([_PERM512, 512 + _PERM512])]
        kp = kr[np.concatenate([_PERM512, 512 + _PERM512])]
        wloc = np.concatenate([qp[:512], kp[:512], vr[:512],
                               qp[512:], kp[512:], vr[512:]],
                              axis=0).astype(np.float64)       # [3DL, D]
        w64h = _f8(64.0 * wloc)
        w1024h = (w64h.astype(np.float64) * 16.0).astype(F8NP)  # exact shift
        w16l = _f8(16.0 * (64.0 * wloc - w64h.astype(np.float64)))
        wAh = np.ascontiguousarray(w1024h.T)                    # [D, 3DL]
        w64hT, w16lT = w64h.T, w16l.T                           # [D, 3DL]
        wBh = np.ascontiguousarray(np.concatenate(
            [np.stack([w64hT[:, c2 * 512:(c2 + 1) * 512],
                       w16lT[:, c2 * 512:(c2 + 1) * 512]],
                      axis=1).reshape(D, 1024)
             for c2 in range(NCH)], axis=1))                    # [D, 6DL]
        woT64 = 64.0 * w_o[:, g * DL:(g + 1) * DL].T.astype(np.float64)
        wo64h = _f8(woT64)
        woAh = np.ascontiguousarray(
            (wo64h.astype(np.float64) * 16.0).astype(F8NP))      # [DL, L]
        wo16l = _f8(16.0 * (woT64 - wo64h.astype(np.float64)))
        woBh = np.ascontiguousarray(
            np.concatenate([wo64h[:, None, :], wo16l[:, None, :]],
                           axis=1).reshape(DL, 2 * L))
        in_maps.append({
            "xThi": xhi_all[b],
            "xTlo": xlo_all[b],
            "wA": wAh,
            "wB": wBh,
            "woA": woAh,
            "woB": woBh,
            "chalf": chalf,
            "shalf": shalf,
            "tri": tri,
        })
    return in_maps


def kernel(x, w_qkv, w_o, _trace=False):
    x = np.asarray(x, dtype=np.float32)
    w_qkv = np.asarray(w_qkv, dtype=np.float32)
    w_o = np.asarray(w_o, dtype=np.float32)
    nc = _get_program()
    in_maps = _host_inputs(x, w_qkv, w_o)
    res = run_bass_kernel_spmd(nc, in_maps, core_ids=list(range(8)),
                               trace=_trace)
    kernel.last_result = res
    parts = [r["outT"].astype(np.float32) for r in res.results]
    out = np.empty((B, L, D), dtype=np.float32)
    for b in range(B):
        out[b] = (parts[b] + parts[b + 4]).T
    return out


# revision 7
# speedup vs baseline: 1.0010x; 1.0010x over previous
"""MHA (RoPE + causal softmax attention + out-proj) on 8 NeuronCores — v3.

Sharding: DP4 x TP2 (core c: batch c % 4, head-group c // 4; 8 heads/core).
Host sums the two head-group partial outputs per batch and transposes.

Key structure (tuned against the TimelineSim cost model):
  * Phase A (QKV) matmuls run in fp8 e4m3 DoubleRow with a hi/lo split of
    both operands (x ~ xh + xl/16, 64w ~ wh + wl/16, lo*lo dropped):
    3 DR instructions per d-tile pair = 0.75x the bf16 instruction cost at
    better-than-bf16 accuracy. PSUM carries 1024*qkv; the 2^-10 unscale is
    folded into the RoPE-cast / v copy scales.
  * q/k comps are host-permuted to (evens | odds) within each head so RoPE
    reads contiguous PSUM blocks: 2 ACT casts + 6 DVE bf16 ops (4x mode).
  * Softmax: exp(alpha*s - 8ln2) -> fp16 exp tiles; denominator accumulated
    with DVE adds + ONE ones-matmul per (head, qc) instead of a ones-matmul
    per k-tile. Causal diagonal tiles are column-trimmed; a single [128,128]
    triangle mask remains.
  * v never round-trips DRAM (PSUM -> SBUF fp16 copy, resident).
  * Emission interleaves head-group-1 QKV tiles into the attention loop of
    head-group-0 so the exp-bound stretch of attention overlaps the
    PE-bound QKV GEMM instead of stalling the tensor engine.
"""

import numpy as np
import ml_dtypes

import concourse.bass as bass
import concourse.tile as tile
import concourse.mybir as mybir
from concourse import bacc
from concourse.bass_utils import run_bass_kernel_spmd

BF16 = ml_dtypes.bfloat16
F8NP = ml_dtypes.float8_e4m3
F32 = mybir.dt.float32
BF = mybir.dt.bfloat16
F16 = mybir.dt.float16
F8 = mybir.dt.float8e4
DR = mybir.MatmulPerfMode.DoubleRow

B, L, D, H, HD = 4, 2048, 2048, 16, 128
NH = 8                      # heads per core
DL = NH * HD                # 1024 local head dims
ROPE_BASE = 10000.0
ALPHA = float(HD) ** -0.5
EXP_BIAS = -8.0 * float(np.log(2.0))   # exp(a*s - 8ln2): keeps fp16 sums safe

LT = L // 128               # 16 L-tiles
DT = D // 128               # 16 D(contract)-tiles
NCH = 6                     # qkv chunks of 512 comps: q03,k03,v03,q47,k47,v47
QC = L // 512               # 4 q-chunks of 512
KT = L // 128               # 16 k-tiles


def _chunk_kind(c):
    # chunk order: q(heads0-3), k(0-3), v(0-3), q(4-7), k(4-7), v(4-7)
    return ("q", "k", "v")[c % 3], c // 3


def build_program(phases="ABC", la=3, scb=3, ypb=1, psab=3, patb=1, paob=3,
                  pbab=4, take=4, takes=None):
    nc = bacc.Bacc("TRN2", target_bir_lowering=False, debug=False, num_devices=8)

    # x hi/lo fp8 planes: hi = fp8(x), lo = fp8(16*(x-hi))
    xThi = nc.dram_tensor("xThi", [D, L], F8, kind="ExternalInput").ap()
    xTlo = nc.dram_tensor("xTlo", [D, L], F8, kind="ExternalInput").ap()
    # w planes: wA = fp8(16*w64_hi) [D, 3DL]; wB [D, 2*3DL]: per 512-chunk,
    # 1024 cols = (w64_hi 512 | w16_lo 512)
    wA = nc.dram_tensor("wA", [D, 3 * DL], F8, kind="ExternalInput").ap()
    wB = nc.dram_tensor("wB", [D, 6 * DL], F8, kind="ExternalInput").ap()
    woA = nc.dram_tensor("woA", [DL, L], F8, kind="ExternalInput").ap()
    woB = nc.dram_tensor("woB", [DL, 2 * L], F8, kind="ExternalInput").ap()
    chalf = nc.dram_tensor("chalf", [L, 256], BF, kind="ExternalInput").ap()
    shalf = nc.dram_tensor("shalf", [L, 256], BF, kind="ExternalInput").ap()
    tri = nc.dram_tensor("tri", [128, 128], F16, kind="ExternalInput").ap()
    outT = nc.dram_tensor("outT", [D, L], BF, kind="ExternalOutput").ap()

    # DRAM staging for rotated q/k, split per head-group so group-0 attention
    # does not depend on group-1 writes
    qkrot = [[nc.dram_tensor(f"{nm}rot{g}", [L, 512], BF, kind="Internal").ap()
              for g in range(2)] for nm in ("q", "k")]

    doA = "A" in phases
    doB = "B" in phases
    doC = "C" in phases

    with tile.TileContext(nc) as tc:
        outer_cm = tc.tile_pool(name="outer", bufs=1)
        pb_cm = tc.tile_pool(name="pBqk", bufs=2, side="right")
        pbm_cm = tc.tile_pool(name="pBm", bufs=1, side="right")
        pby03_cm = tc.tile_pool(name="pBy03", bufs=1)
        pba_cm = tc.tile_pool(name="pBa", bufs=pbab)
        pbr_cm = tc.tile_pool(name="pBr", bufs=1)
        pbv_cm = tc.tile_pool(name="pBv", bufs=1)
        pbd_cm = tc.tile_pool(name="pBd", bufs=2)
        P = {}
        with outer_cm as outer, pb_cm as pb, pbm_cm as pbm, \
             pby03_cm as pby03, pba_cm as pba, pbr_cm as pbr, \
             pbv_cm as pbv, pbd_cm as pbd:
            # v for both head groups, resident SBUF: [128(kpos), LT, 512]
            vsb = [outer.tile([128, LT, 512], F16, name=f"vsb{g}",
                              tag=f"vsb{g}") for g in range(2)]
            ones128 = outer.tile([128, 128], F16, name="ones128", tag="oc")
            nc.vector.memset(ones128, 1.0)
            ebias = outer.tile([128, 1], F32, name="ebias", tag="ebias")
            nc.vector.memset(ebias, EXP_BIAS)
            trit = pbm.tile([128, 128], F16, name="tri", tag="tri")
            nc.sync.dma_start(out=trit, in_=tri)

            qts = {}
            yall = {}
            yall[0] = pby03.tile([128, 4, 2, L], F8, name="yall0", tag="yall0")
            if not doB:
                nc.vector.memset(yall[0], 0.0)

            # ---------------- phase B helpers ----------------
            def load_qk(h):
                grp, hh = h // 4, h % 4
                qt = pb.tile([128, L], BF, name="qt", tag="qt")
                kt = pb.tile([128, L], BF, name="kt", tag="kt")
                for t, src in ((qt, qkrot[0][grp]), (kt, qkrot[1][grp])):
                    nc.sync.dma_start_transpose(
                        out=t, in_=src[:, hh * 128:(hh + 1) * 128])
                return qt, kt

            def emit_B_qc(h, qt, kt, qc):
                grp, hh = h // 4, h % 4
                nkt = 4 * qc + 4
                ypsum = P["psy"].tile([128, 512], F32, name="ypsum", tag="yp")
                dacc = pbd.tile([128, 512], F16, name="dacc", tag="dacc")
                ats = {}

                def emit_score(j):
                    m = j - 4 * qc
                    off = 128 * m if m > 0 else 0
                    w = 512 - off
                    sc = P["pss"].tile([128, 512], F32, name="sc", tag="sc")
                    nc.tensor.matmul(
                        sc[:, 0:w], kt[:, j * 128:(j + 1) * 128],
                        qt[:, qc * 512 + off:(qc + 1) * 512],
                        start=True, stop=True)
                    at = pba.tile([128, 512], F16, name="at", tag="at")
                    nc.scalar.activation(
                        out=at[:, 0:w], in_=sc[:, 0:w],
                        func=mybir.ActivationFunctionType.Exp,
                        scale=ALPHA, bias=ebias)
                    if m >= 0:
                        nc.vector.tensor_mul(at[:, 0:128], at[:, 0:128], trit)
                    ats[j] = (at, off, w)

                for j in range(min(la, nkt)):
                    emit_score(j)
                prev = None
                for j in range(nkt):
                    if j + la < nkt:
                        emit_score(j + la)
                    at, off, w = ats.pop(j)
                    nc.tensor.matmul(
                        ypsum[:, off:512],
                        vsb[grp][:, j, hh * 128:(hh + 1) * 128],
                        at[:, 0:w],
                        start=(j == 0), stop=(j == nkt - 1),
                        skip_group_check=True)
                    if j == 0:
                        prev = at
                    elif j == 1:
                        if qc == 0:
                            nc.vector.tensor_copy(
                                out=dacc[:, 0:128], in_=prev[:, 0:128])
                            nc.vector.tensor_add(
                                dacc[:, 128:512], prev[:, 128:512],
                                at[:, 0:w])
                        else:
                            nc.vector.tensor_add(dacc, prev, at)
                    else:
                        nc.vector.tensor_add(
                            dacc[:, off:512], dacc[:, off:512], at[:, 0:w])
                dpsum = P["psd"].tile([128, 512], F32, name="dpsum", tag="dp")
                nc.tensor.matmul(dpsum, ones128, dacc, start=True, stop=True)
                rbs = pbr.tile([128, 512], BF, name="rbs", tag="rbs")
                with nc.allow_low_precision("softmax recip bf16"):
                    nc.vector.reciprocal(out=rbs, in_=dpsum)
                qsl = slice(qc * 512, (qc + 1) * 512)
                ya = yall[grp]
                yb = pbv.tile([128, 512], BF, name="yb", tag="yb")
                nc.vector.tensor_mul(yb, ypsum, rbs)
                nc.gpsimd.tensor_copy(out=ya[:, hh, 1, qsl], in_=yb)
                yd = pbv.tile([128, 512], BF, name="yd", tag="yd")
                nc.gpsimd.tensor_sub(yd, yb, ya[:, hh, 1, qsl])
                nc.gpsimd.tensor_scalar_mul(ya[:, hh, 0, qsl], yd, 16.0)

            # ---------------- phase A scope + interleave ----------------
            with tc.tile_pool(name="pA", bufs=1) as pa, \
                 tc.tile_pool(name="pAw", bufs=2) as paw, \
                 tc.tile_pool(name="pAt", bufs=patb) as pat, \
                 tc.tile_pool(name="pAo", bufs=paob) as pao:
                xall = pa.tile([128, DT, 2, L], F8, name="xall", tag="xall")
                c_sb = pa.tile([128, LT, 256], BF, name="c_sb", tag="c_sb")
                s_sb = pa.tile([128, LT, 256], BF, name="s_sb", tag="s_sb")
                wch = {}

                def load_wch(c):
                    if c >= NCH or c in wch:
                        return
                    wa = paw.tile([128, DT, 512], F8, name="wchA", tag="wchA")
                    wb = paw.tile([128, DT, 2, 512], F8, name="wchB",
                                  tag="wchB")
                    wAr = wA[:, c * 512:(c + 1) * 512].rearrange(
                        "(d p) e -> p d e", p=128)
                    wBr = wB[:, c * 1024:(c + 1) * 1024].rearrange(
                        "(d p) e -> p d e", p=128)
                    wbf = wb.rearrange("p d t e -> p d (t e)")
                    for d4 in range(DT // 4):
                        sl = slice(4 * d4, 4 * d4 + 4)
                        nc.sync.dma_start(out=wa[:, sl, :], in_=wAr[:, sl, :])
                        nc.sync.dma_start(out=wbf[:, sl, :], in_=wBr[:, sl, :])
                    wch[c] = (wa, wb)

                def load_x():
                    wa = paw.tile([128, DT, 512], F8, name="wchA", tag="wchA")
                    wb = paw.tile([128, DT, 2, 512], F8, name="wchB",
                                  tag="wchB")
                    wAr = wA[:, 0:512].rearrange("(d p) e -> p d e", p=128)
                    wBr = wB[:, 0:1024].rearrange("(d p) e -> p d e", p=128)
                    wbf = wb.rearrange("p d t e -> p d (t e)")
                    xhr = xThi.rearrange("(d p) l -> p d l", p=128)
                    xlr = xTlo.rearrange("(d p) l -> p d l", p=128)
                    for d2 in range(DT // 2):
                        sl = slice(2 * d2, 2 * d2 + 2)
                        nc.sync.dma_start(out=xall[:, sl, 1, :],
                                          in_=xhr[:, sl, :])
                        nc.sync.dma_start(out=xall[:, sl, 0, :],
                                          in_=xlr[:, sl, :])
                        if d2 % 2 == 1:
                            sl4 = slice(2 * d2 - 2, 2 * d2 + 2)
                            nc.sync.dma_start(out=wa[:, sl4, :],
                                              in_=wAr[:, sl4, :])
                            nc.sync.dma_start(out=wbf[:, sl4, :],
                                              in_=wBr[:, sl4, :])
                            i2 = slice(d2 - 1, d2 + 1)
                            for t_sb, t_dr in ((c_sb, chalf), (s_sb, shalf)):
                                nc.sync.dma_start(
                                    out=t_sb[:, i2, :],
                                    in_=t_dr.rearrange("(i p) g -> p i g",
                                                       p=128)[:, i2, :])
                    # rope tables for the second half of the l range trail
                    for t_sb, t_dr in ((c_sb, chalf), (s_sb, shalf)):
                        nc.sync.dma_start(
                            out=t_sb[:, 8:LT, :],
                            in_=t_dr.rearrange("(i p) g -> p i g",
                                               p=128)[:, 8:LT, :])
                    wch[0] = (wa, wb)

                def emit_A_alpha(c, i):
                    wa, _ = wch[c]
                    ls = slice(i * 128, (i + 1) * 128)
                    pnat = P["psa"].tile([128, 512], F32, name="pnat",
                                         tag="pnat")
                    for d2 in range(DT // 2):
                        nc.tensor.matmul(
                            pnat,
                            xall[:, 2 * d2:2 * d2 + 2, 1, ls],
                            wa[:, 2 * d2:2 * d2 + 2, :],
                            start=(d2 == 0), stop=False, perf_mode=DR)
                    return pnat

                def emit_A_finish(c, i, pnat):
                    _, wb = wch[c]
                    if i == 8:
                        load_wch(c + 1)
                    ls = slice(i * 128, (i + 1) * 128)
                    for d in range(DT):
                        nc.tensor.matmul(
                            pnat,
                            xall[:, d, :, ls],
                            wb[:, d, :, :],
                            start=False, stop=(d == DT - 1), perf_mode=DR)
                    emit_A_rope(c, i, pnat)

                def emit_A_rope(c, i, pnat):
                    kind, grp = _chunk_kind(c)
                    ls = slice(i * 128, (i + 1) * 128)
                    if kind == "v":
                        nc.scalar.activation(
                            out=vsb[grp][:, i, :], in_=pnat,
                            func=mybir.ActivationFunctionType.Copy,
                            scale=1.0 / 1024.0)
                        return
                    # RoPE: per-head comps are permuted (evens | odds)
                    pv = pnat.rearrange("p (hh t z) -> p hh t z",
                                        hh=4, t=2, z=64)
                    x1 = pat.tile([128, 256], BF, name="x1", tag="x1")
                    nc.scalar.activation(
                        out=x1, in_=pv[:, :, 0, :],
                        func=mybir.ActivationFunctionType.Copy,
                        scale=1.0 / 1024.0)
                    x2 = pat.tile([128, 256], BF, name="x2", tag="x2")
                    nc.scalar.activation(
                        out=x2, in_=pv[:, :, 1, :],
                        func=mybir.ActivationFunctionType.Copy,
                        scale=1.0 / 1024.0)
                    ct = c_sb[:, i, :]
                    st = s_sb[:, i, :]
                    t1 = pat.tile([128, 256], BF, name="t1", tag="t1")
                    nc.vector.tensor_mul(t1, x1, ct)
                    t2 = pat.tile([128, 256], BF, name="t2", tag="t2")
                    nc.vector.tensor_mul(t2, x2, st)
                    t3 = pat.tile([128, 256], BF, name="t3", tag="t3")
                    nc.vector.tensor_mul(t3, x2, ct)
                    t4 = pat.tile([128, 256], BF, name="t4", tag="t4")
                    nc.vector.tensor_mul(t4, x1, st)
                    ro = pao.tile([128, 512], BF, name="ro", tag="ro")
                    rv = ro.rearrange("p (hh t z) -> p hh t z", hh=4, t=2, z=64)
                    nc.vector.tensor_sub(rv[:, :, 0, :], t1, t2)
                    nc.vector.tensor_add(rv[:, :, 1, :], t3, t4)
                    dst = qkrot[0 if kind == "q" else 1][grp]
                    nc.sync.dma_start(out=dst[ls, :], in_=ro)

                def emit_A_tile(c, i):
                    emit_A_finish(c, i, emit_A_alpha(c, i))

                def emit_A_dmajor(c, tiles):
                    # d-major across several open psum groups: every arriving
                    # x/w piece-group unlocks one alpha+2 betas per open tile
                    wa, wb = wch[c]
                    pns = [P["psa"].tile([128, 512], F32, name="pnat",
                                         tag="pnat") for _ in tiles]
                    for d2 in range(DT // 2):
                        for t, i in enumerate(tiles):
                            ls = slice(i * 128, (i + 1) * 128)
                            nc.tensor.matmul(
                                pns[t],
                                xall[:, 2 * d2:2 * d2 + 2, 1, ls],
                                wa[:, 2 * d2:2 * d2 + 2, :],
                                start=(d2 == 0), stop=False, perf_mode=DR)
                        for dd in (2 * d2, 2 * d2 + 1):
                            for t, i in enumerate(tiles):
                                ls = slice(i * 128, (i + 1) * 128)
                                nc.tensor.matmul(
                                    pns[t],
                                    xall[:, dd, :, ls],
                                    wb[:, dd, :, :],
                                    start=False, stop=(dd == DT - 1),
                                    perf_mode=DR)
                    for t, i in enumerate(tiles):
                        emit_A_rope(c, i, pns[t])

                # ---- emission: A(g0), then heads 0-3 x A(g1) ----
                if doA:
                    load_x()
                    # chunk 0 lead-in: 7 psum banks, d-major
                    with tc.tile_pool(name="psA0", bufs=8,
                                      space="PSUM") as psa0:
                        P["psa"] = psa0
                        emit_A_dmajor(0, range(8))
                        for i in range(8, LT):
                            emit_A_tile(0, i)
                        for c in (1, 2):
                            for i in range(LT):
                                emit_A_tile(c, i)
                with tc.tile_pool(name="psS", bufs=scb, space="PSUM") as pss, \
                     tc.tile_pool(name="psY", bufs=ypb, space="PSUM") as psy, \
                     tc.tile_pool(name="psD", bufs=1, space="PSUM") as psd:
                    P["pss"], P["psy"], P["psd"] = pss, psy, psd
                    with tc.tile_pool(name="psA", bufs=psab,
                                      space="PSUM") as psa:
                        P["psa"] = psa
                        g1 = [(c, i) for c in (3, 4, 5) for i in range(LT)] \
                            if doA else []
                        gi = 0
                        qts.clear()
                        slot = 0
                        for h in range(3 if doB else 0):
                            if h not in qts:
                                qts[h] = load_qk(h)
                            qt, kt = qts.pop(h)
                            for qc in range(QC):
                                emit_B_qc(h, qt, kt, qc)
                                if qc == 0 and h < 3 and doB:
                                    qts[h + 1] = load_qk(h + 1)
                                tk = takes[slot] if takes else take
                                slot += 1
                                for _ in range(tk):
                                    if gi < len(g1):
                                        emit_A_tile(*g1[gi])
                                        gi += 1
                        while gi < len(g1):
                            emit_A_tile(*g1[gi])
                            gi += 1

            # ---------------- heads 4-7 (qc-major) + phase C ----------
            with tc.tile_pool(name="pBy47", bufs=1) as pby47, \
                 tc.tile_pool(name="pCo", bufs=4) as pco, \
                 tc.tile_pool(name="psS3", bufs=scb, space="PSUM") as pss3, \
                 tc.tile_pool(name="psY3", bufs=ypb, space="PSUM") as psy3, \
                 tc.tile_pool(name="psD3", bufs=1, space="PSUM") as psd3, \
                 tc.tile_pool(name="psC", bufs=2, space="PSUM") as psc:
                P["pss"], P["psy"], P["psd"] = pss3, psy3, psd3
                qk47 = dict(qts)   # h3 was prefetched into the pb pool
                for h in range(4, NH if doB else 4):
                    grp, hh = h // 4, h % 4
                    qt = pby47.tile([128, L], BF, name=f"qt{h}", tag=f"qt{h}")
                    kt = pby47.tile([128, L], BF, name=f"kt{h}", tag=f"kt{h}")
                    for t, src in ((qt, qkrot[0][grp]), (kt, qkrot[1][grp])):
                        nc.sync.dma_start_transpose(
                            out=t, in_=src[:, hh * 128:(hh + 1) * 128])
                    qk47[h] = (qt, kt)
                yall[1] = pby47.tile([128, 4, 2, L], F8, name="yall1",
                                     tag="yall1")
                if not doB:
                    nc.vector.memset(yall[1], 0.0)
                woat = pby47.tile([128, NH, L], F8, name="woat", tag="woat")
                nc.sync.dma_start(
                    out=woat, in_=woA.rearrange("(dd p) e -> p dd e", p=128))
                wobt = pby47.tile([128, NH, 2, L], F8, name="wobt", tag="wobt")
                nc.sync.dma_start(
                    out=wobt.rearrange("p dd t e -> p dd (t e)"),
                    in_=woB.rearrange("(dd p) e -> p dd e", p=128))

                def emit_C(e, qc):
                    esl = slice(e * 128, (e + 1) * 128)
                    qsl = slice(qc * 512, (qc + 1) * 512)
                    op = psc.tile([128, 512], F32, name="op", tag="op")
                    for g in range(2):
                        for p2 in range(2):
                            dd = 4 * g + 2 * p2
                            nc.tensor.matmul(
                                op, woat[:, dd:dd + 2, esl],
                                yall[g][:, 2 * p2:2 * p2 + 2, 1, qsl],
                                start=(g == 0 and p2 == 0), stop=False,
                                perf_mode=DR)
                    for g in range(2):
                        for hh in range(4):
                            nc.tensor.matmul(
                                op, wobt[:, 4 * g + hh, :, esl],
                                yall[g][:, hh, :, qsl],
                                start=False, stop=(g == 1 and hh == 3),
                                perf_mode=DR)
                    ot = pco.tile([128, 512], BF, name="ot", tag="ot")
                    if (e + qc) % 2 == 0:
                        nc.scalar.activation(
                            out=ot, in_=op,
                            func=mybir.ActivationFunctionType.Copy,
                            scale=1.0 / 1024.0)
                    else:
                        nc.vector.tensor_scalar_mul(ot, op, 1.0 / 1024.0)
                    nc.sync.dma_start(
                        out=outT[e * 128:(e + 1) * 128,
                                 qc * 512:(qc + 1) * 512],
                        in_=ot)

                for qc in range(QC if doB else 0):
                    for h in range(3, NH):
                        emit_B_qc(h, qk47[h][0], qk47[h][1], qc)
                    if doC and qc > 0:
                        for e in range(DT):
                            emit_C(e, qc - 1)
                if doC:
                    for qc in ([3] if doB else range(QC)):
                        for e in range(DT):
                            emit_C(e, qc)
    nc.compile()
    return nc


_NC_CACHE = None


def _get_program():
    global _NC_CACHE
    if _NC_CACHE is None:
        _NC_CACHE = build_program()
    return _NC_CACHE


def _f8(a):
    return np.clip(np.asarray(a, np.float64), -240.0, 240.0).astype(F8NP)


# within each head's 128 comps: evens first, then odds
_PERM512 = np.concatenate(
    [np.concatenate([np.arange(h * 128, (h + 1) * 128, 2),
                     np.arange(h * 128 + 1, (h + 1) * 128, 2)])
     for h in range(4)])


def _host_inputs(x, w_qkv, w_o):
    inv = 1.0 / (ROPE_BASE ** (np.arange(0, HD, 2, dtype=np.float64) / HD))
    ang = np.arange(L, dtype=np.float64)[:, None] * inv[None, :]
    chalf = np.tile(np.cos(ang), (1, 4)).astype(BF16)          # [L, 256]
    shalf = np.tile(np.sin(ang), (1, 4)).astype(BF16)
    p = np.arange(128)[:, None]
    f = np.arange(128)[None, :]
    tri = (p <= f).astype(np.float16)                          # [128, 128]

    xhi_all, xlo_all = [], []
    for b in range(B):
        xb = np.asarray(x[b], np.float64)                      # [L, D]
        xh = _f8(xb)
        xl = _f8(16.0 * (xb - xh.astype(np.float64)))
        xhi_all.append(np.ascontiguousarray(xh.T))
        xlo_all.append(np.ascontiguousarray(xl.T))

    in_maps = []
    for c in range(8):
        b, g = c % 4, c // 4
        qr = w_qkv[g * DL:(g + 1) * DL]
        kr = w_qkv[D + g * DL:D + (g + 1) * DL]
        vr = w_qkv[2 * D + g * DL:2 * D + (g + 1) * DL]
        qp = qr[np.concatenate([_PERM512, 512 + _PERM512])]
        kp = kr[np.concatenate([_PERM512, 512 + _PERM512])]
        wloc = np.concatenate([qp[:512], kp[:512], vr[:512],
                               qp[512:], kp[512:], vr[512:]],
                              axis=0).astype(np.float64)       # [3DL, D]
        w64h = _f8(64.0 * wloc)
        w1024h = (w64h.astype(np.float64) * 16.0).astype(F8NP)  # exact shift
        w16l = _f8(16.0 * (64.0 * wloc - w64h.astype(np.float64)))
        wAh = np.ascontiguousarray(w1024h.T)                    # [D, 3DL]
        w64hT, w16lT = w64h.T, w16l.T                           # [D, 3DL]
        wBh = np.ascontiguousarray(np.concatenate(
            [np.stack([w64hT[:, c2 * 512:(c2 + 1) * 512],
                       w16lT[:, c2 * 512:(c2 + 1) * 512]],
                      axis=1).reshape(D, 1024)
             for c2 in range(NCH)], axis=1))                    # [D, 6DL]
        woT64 = 64.0 * w_o[:, g * DL:(g + 1) * DL].T.astype(np.float64)
        wo64h = _f8(woT64)
        woAh = np.ascontiguousarray(
            (wo64h.astype(np.float64) * 16.0).astype(F8NP))      # [DL, L]
        wo16l = _f8(16.0 * (woT64 - wo64h.astype(np.float64)))
        woBh = np.ascontiguousarray(
            np.concatenate([wo64h[:, None, :], wo16l[:, None, :]],
                           axis=1).reshape(DL, 2 * L))
        in_maps.append({
            "xThi": xhi_all[b],
            "xTlo": xlo_all[b],
            "wA": wAh,
            "wB": wBh,
            "woA": woAh,
            "woB": woBh,
            "chalf": chalf,
            "shalf": shalf,
            "tri": tri,
        })
    return in_maps


def kernel(x, w_qkv, w_o, _trace=False):
    x = np.asarray(x, dtype=np.float32)
    w_qkv = np.asarray(w_qkv, dtype=np.float32)
    w_o = np.asarray(w_o, dtype=np.float32)
    nc = _get_program()
    in_maps = _host_inputs(x, w_qkv, w_o)
    res = run_bass_kernel_spmd(nc, in_maps, core_ids=list(range(8)),
                               trace=_trace)
    kernel.last_result = res
    parts = [r["outT"].astype(np.float32) for r in res.results]
    out = np.empty((B, L, D), dtype=np.float32)
    for b in range(B):
        out[b] = (parts[b] + parts[b + 4]).T
    return out


# revision 9
# speedup vs baseline: 1.0060x; 1.0049x over previous
"""MHA (RoPE + causal softmax attention + out-proj) on 8 NeuronCores — v3.

Sharding: DP4 x TP2 (core c: batch c % 4, head-group c // 4; 8 heads/core).
Host sums the two head-group partial outputs per batch and transposes.

Key structure (tuned against the TimelineSim cost model):
  * Phase A (QKV) matmuls run in fp8 e4m3 DoubleRow with a hi/lo split of
    both operands (x ~ xh + xl/16, 64w ~ wh + wl/16, lo*lo dropped):
    3 DR instructions per d-tile pair = 0.75x the bf16 instruction cost at
    better-than-bf16 accuracy. PSUM carries 1024*qkv; the 2^-10 unscale is
    folded into the RoPE-cast / v copy scales.
  * q/k comps are host-permuted to (evens | odds) within each head so RoPE
    reads contiguous PSUM blocks: 2 ACT casts + 6 DVE bf16 ops (4x mode).
  * Softmax: exp(alpha*s - 8ln2) -> fp16 exp tiles; denominator accumulated
    with DVE adds + ONE ones-matmul per (head, qc) instead of a ones-matmul
    per k-tile. Causal diagonal tiles are column-trimmed; a single [128,128]
    triangle mask remains.
  * v never round-trips DRAM (PSUM -> SBUF fp16 copy, resident).
  * Emission interleaves head-group-1 QKV tiles into the attention loop of
    head-group-0 so the exp-bound stretch of attention overlaps the
    PE-bound QKV GEMM instead of stalling the tensor engine.
"""

import numpy as np
import ml_dtypes

import concourse.bass as bass
import concourse.tile as tile
import concourse.mybir as mybir
from concourse import bacc
from concourse.bass_utils import run_bass_kernel_spmd

BF16 = ml_dtypes.bfloat16
F8NP = ml_dtypes.float8_e4m3
F32 = mybir.dt.float32
BF = mybir.dt.bfloat16
F16 = mybir.dt.float16
F8 = mybir.dt.float8e4
DR = mybir.MatmulPerfMode.DoubleRow

B, L, D, H, HD = 4, 2048, 2048, 16, 128
NH = 8                      # heads per core
DL = NH * HD                # 1024 local head dims
ROPE_BASE = 10000.0
ALPHA = float(HD) ** -0.5
EXP_BIAS = -8.0 * float(np.log(2.0))   # exp(a*s - 8ln2): keeps fp16 sums safe

LT = L // 128               # 16 L-tiles
DT = D // 128               # 16 D(contract)-tiles
NCH = 6                     # qkv chunks of 512 comps: q03,k03,v03,q47,k47,v47
QC = L // 512               # 4 q-chunks of 512
KT = L // 128               # 16 k-tiles


def _chunk_kind(c):
    # chunk order: q(heads0-3), k(0-3), v(0-3), q(4-7), k(4-7), v(4-7)
    return ("q", "k", "v")[c % 3], c // 3


def build_program(phases="ABC", la=3, scb=3, ypb=1, psab=3, patb=1, paob=3,
                  pbab=5, take=4, takes=None):
    nc = bacc.Bacc("TRN2", target_bir_lowering=False, debug=False, num_devices=8)

    # x hi/lo fp8 planes: hi = fp8(x), lo = fp8(16*(x-hi))
    xThi = nc.dram_tensor("xThi", [D, L], F8, kind="ExternalInput").ap()
    xTlo = nc.dram_tensor("xTlo", [D, L], F8, kind="ExternalInput").ap()
    # w planes: wA = fp8(16*w64_hi) [D, 3DL]; wB [D, 2*3DL]: per 512-chunk,
    # 1024 cols = (w64_hi 512 | w16_lo 512)
    wA = nc.dram_tensor("wA", [D, 3 * DL], F8, kind="ExternalInput").ap()
    wB = nc.dram_tensor("wB", [D, 6 * DL], F8, kind="ExternalInput").ap()
    woA = nc.dram_tensor("woA", [DL, L], F8, kind="ExternalInput").ap()
    woB = nc.dram_tensor("woB", [DL, 2 * L], F8, kind="ExternalInput").ap()
    chalf = nc.dram_tensor("chalf", [L, 64], BF, kind="ExternalInput").ap()
    shalf = nc.dram_tensor("shalf", [L, 64], BF, kind="ExternalInput").ap()
    tri = nc.dram_tensor("tri", [128, 128], F16, kind="ExternalInput").ap()
    outT = nc.dram_tensor("outT", [D, L], BF, kind="ExternalOutput").ap()

    # DRAM staging for rotated q/k, split per head-group so group-0 attention
    # does not depend on group-1 writes
    qkrot = [[nc.dram_tensor(f"{nm}rot{g}", [L, 512], BF, kind="Internal").ap()
              for g in range(2)] for nm in ("q", "k")]

    doA = "A" in phases
    doB = "B" in phases
    doC = "C" in phases

    with tile.TileContext(nc) as tc:
        outer_cm = tc.tile_pool(name="outer", bufs=1)
        pb_cm = tc.tile_pool(name="pBqk", bufs=2, side="right")
        pbm_cm = tc.tile_pool(name="pBm", bufs=1, side="right")
        pby03_cm = tc.tile_pool(name="pBy03", bufs=1)
        pba_cm = tc.tile_pool(name="pBa", bufs=pbab)
        pbr_cm = tc.tile_pool(name="pBr", bufs=1)
        pbv_cm = tc.tile_pool(name="pBv", bufs=1)
        pbd_cm = tc.tile_pool(name="pBd", bufs=2)
        P = {}
        with outer_cm as outer, pb_cm as pb, pbm_cm as pbm, \
             pby03_cm as pby03, pba_cm as pba, pbr_cm as pbr, \
             pbv_cm as pbv, pbd_cm as pbd:
            # v for both head groups, resident SBUF: [128(kpos), LT, 512]
            vsb = [outer.tile([128, LT, 512], F16, name=f"vsb{g}",
                              tag=f"vsb{g}") for g in range(2)]
            ones128 = outer.tile([128, 128], F16, name="ones128", tag="oc")
            nc.vector.memset(ones128, 1.0)
            ebias = outer.tile([128, 1], F32, name="ebias", tag="ebias")
            nc.vector.memset(ebias, EXP_BIAS)
            trit = pbm.tile([128, 128], F16, name="tri", tag="tri")
            nc.sync.dma_start(out=trit, in_=tri)

            qts = {}
            yall = {}
            yall[0] = pby03.tile([128, 4, 2, L], F8, name="yall0", tag="yall0")
            if not doB:
                nc.vector.memset(yall[0], 0.0)

            # ---------------- phase B helpers ----------------
            def load_qk(h):
                grp, hh = h // 4, h % 4
                qt = pb.tile([128, L], BF, name="qt", tag="qt")
                kt = pb.tile([128, L], BF, name="kt", tag="kt")
                for t, src in ((qt, qkrot[0][grp]), (kt, qkrot[1][grp])):
                    nc.sync.dma_start_transpose(
                        out=t, in_=src[:, hh * 128:(hh + 1) * 128])
                return qt, kt

            def emit_B_qc(h, qt, kt, qc):
                grp, hh = h // 4, h % 4
                nkt = 4 * qc + 4
                ypsum = P["psy"].tile([128, 512], F32, name="ypsum", tag="yp")
                dacc = pbd.tile([128, 512], F16, name="dacc", tag="dacc")
                ats = {}

                def emit_score(j):
                    m = j - 4 * qc
                    off = 128 * m if m > 0 else 0
                    w = 512 - off
                    sc = P["pss"].tile([128, 512], F32, name="sc", tag="sc")
                    nc.tensor.matmul(
                        sc[:, 0:w], kt[:, j * 128:(j + 1) * 128],
                        qt[:, qc * 512 + off:(qc + 1) * 512],
                        start=True, stop=True)
                    at = pba.tile([128, 512], F16, name="at", tag="at")
                    nc.scalar.activation(
                        out=at[:, 0:w], in_=sc[:, 0:w],
                        func=mybir.ActivationFunctionType.Exp,
                        scale=ALPHA, bias=ebias)
                    if m >= 0:
                        nc.vector.tensor_mul(at[:, 0:128], at[:, 0:128], trit)
                    ats[j] = (at, off, w)

                for j in range(min(la, nkt)):
                    emit_score(j)
                prev = None
                for j in range(nkt):
                    if j + la < nkt:
                        emit_score(j + la)
                    at, off, w = ats.pop(j)
                    nc.tensor.matmul(
                        ypsum[:, off:512],
                        vsb[grp][:, j, hh * 128:(hh + 1) * 128],
                        at[:, 0:w],
                        start=(j == 0), stop=(j == nkt - 1),
                        skip_group_check=True)
                    if j == 0:
                        prev = at
                    elif j == 1:
                        if qc == 0:
                            nc.vector.tensor_copy(
                                out=dacc[:, 0:128], in_=prev[:, 0:128])
                            nc.vector.tensor_add(
                                dacc[:, 128:512], prev[:, 128:512],
                                at[:, 0:w])
                        else:
                            nc.vector.tensor_add(dacc, prev, at)
                    else:
                        nc.vector.tensor_add(
                            dacc[:, off:512], dacc[:, off:512], at[:, 0:w])
                dpsum = P["psd"].tile([128, 512], F32, name="dpsum", tag="dp")
                nc.tensor.matmul(dpsum, ones128, dacc, start=True, stop=True)
                rbs = pbr.tile([128, 512], BF, name="rbs", tag="rbs")
                with nc.allow_low_precision("softmax recip bf16"):
                    nc.vector.reciprocal(out=rbs, in_=dpsum)
                qsl = slice(qc * 512, (qc + 1) * 512)
                ya = yall[grp]
                yb = pbv.tile([128, 512], BF, name="yb", tag="yb")
                nc.vector.tensor_mul(yb, ypsum, rbs)
                nc.gpsimd.tensor_copy(out=ya[:, hh, 1, qsl], in_=yb)
                yd = pbv.tile([128, 512], BF, name="yd", tag="yd")
                nc.gpsimd.tensor_sub(yd, yb, ya[:, hh, 1, qsl])
                nc.gpsimd.tensor_scalar_mul(ya[:, hh, 0, qsl], yd, 16.0)

            # ---------------- phase A scope + interleave ----------------
            with tc.tile_pool(name="pA", bufs=1) as pa, \
                 tc.tile_pool(name="pAw", bufs=2) as paw, \
                 tc.tile_pool(name="pAt", bufs=patb) as pat, \
                 tc.tile_pool(name="pAp", bufs=16) as pap, \
                 tc.tile_pool(name="pAo", bufs=paob) as pao:
                xall = pa.tile([128, DT, 2, L], F8, name="xall", tag="xall")
                c_sb = pa.tile([128, LT, 64], BF, name="c_sb", tag="c_sb")
                s_sb = pa.tile([128, LT, 64], BF, name="s_sb", tag="s_sb")
                wch = {}

                def load_wch(c):
                    if c >= NCH or c in wch:
                        return
                    wa = paw.tile([128, DT, 512], F8, name="wchA", tag="wchA")
                    wb = paw.tile([128, DT, 2, 512], F8, name="wchB",
                                  tag="wchB")
                    wAr = wA[:, c * 512:(c + 1) * 512].rearrange(
                        "(d p) e -> p d e", p=128)
                    wBr = wB[:, c * 1024:(c + 1) * 1024].rearrange(
                        "(d p) e -> p d e", p=128)
                    wbf = wb.rearrange("p d t e -> p d (t e)")
                    for d4 in range(DT // 4):
                        sl = slice(4 * d4, 4 * d4 + 4)
                        nc.sync.dma_start(out=wa[:, sl, :], in_=wAr[:, sl, :])
                        nc.sync.dma_start(out=wbf[:, sl, :], in_=wBr[:, sl, :])
                    wch[c] = (wa, wb)

                def load_x():
                    wa = paw.tile([128, DT, 512], F8, name="wchA", tag="wchA")
                    wb = paw.tile([128, DT, 2, 512], F8, name="wchB",
                                  tag="wchB")
                    wAr = wA[:, 0:512].rearrange("(d p) e -> p d e", p=128)
                    wBr = wB[:, 0:1024].rearrange("(d p) e -> p d e", p=128)
                    wbf = wb.rearrange("p d t e -> p d (t e)")
                    xhr = xThi.rearrange("(d p) l -> p d l", p=128)
                    xlr = xTlo.rearrange("(d p) l -> p d l", p=128)
                    for d2 in range(DT // 2):
                        sl = slice(2 * d2, 2 * d2 + 2)
                        nc.sync.dma_start(out=xall[:, sl, 1, :],
                                          in_=xhr[:, sl, :])
                        nc.sync.dma_start(out=xall[:, sl, 0, :],
                                          in_=xlr[:, sl, :])
                        if d2 % 2 == 1:
                            sl4 = slice(2 * d2 - 2, 2 * d2 + 2)
                            nc.sync.dma_start(out=wa[:, sl4, :],
                                              in_=wAr[:, sl4, :])
                            nc.sync.dma_start(out=wbf[:, sl4, :],
                                              in_=wBr[:, sl4, :])
                            if d2 == 1:
                                for t_sb, t_dr in ((c_sb, chalf),
                                                   (s_sb, shalf)):
                                    nc.sync.dma_start(
                                        out=t_sb,
                                        in_=t_dr.rearrange(
                                            "(i p) g -> p i g", p=128))
                    wch[0] = (wa, wb)

                def emit_A_alpha(c, i):
                    wa, _ = wch[c]
                    ls = slice(i * 128, (i + 1) * 128)
                    pnat = P["psa"].tile([128, 512], F32, name="pnat",
                                         tag="pnat")
                    for d2 in range(DT // 2):
                        nc.tensor.matmul(
                            pnat,
                            xall[:, 2 * d2:2 * d2 + 2, 1, ls],
                            wa[:, 2 * d2:2 * d2 + 2, :],
                            start=(d2 == 0), stop=False, perf_mode=DR)
                    return pnat

                def emit_A_finish(c, i, pnat):
                    _, wb = wch[c]
                    if i == 8:
                        load_wch(c + 1)
                    ls = slice(i * 128, (i + 1) * 128)
                    for d in range(DT):
                        nc.tensor.matmul(
                            pnat,
                            xall[:, d, :, ls],
                            wb[:, d, :, :],
                            start=False, stop=(d == DT - 1), perf_mode=DR)
                    emit_A_rope(c, i, pnat)

                def emit_A_rope(c, i, pnat, part=None):
                    kind, grp = _chunk_kind(c)
                    ls = slice(i * 128, (i + 1) * 128)
                    if kind == "v":
                        nc.scalar.activation(
                            out=vsb[grp][:, i, :], in_=pnat,
                            func=mybir.ActivationFunctionType.Copy,
                            scale=1.0 / 1024.0)
                        return
                    # RoPE: per-head comps are permuted (evens | odds)
                    pv = pnat.rearrange("p (hh t z) -> p hh t z",
                                        hh=4, t=2, z=64)
                    x1 = pat.tile([128, 4, 64], BF, name="x1", tag="x1")
                    x2 = pat.tile([128, 4, 64], BF, name="x2", tag="x2")
                    if part is None:
                        nc.scalar.activation(
                            out=x1, in_=pv[:, :, 0, :],
                            func=mybir.ActivationFunctionType.Copy)
                        nc.scalar.activation(
                            out=x2, in_=pv[:, :, 1, :],
                            func=mybir.ActivationFunctionType.Copy)
                    else:
                        pw = part.rearrange("p (hh t z) -> p hh t z",
                                            hh=4, t=2, z=64)
                        nc.vector.tensor_add(x1, pv[:, :, 0, :],
                                             pw[:, :, 0, :])
                        nc.vector.tensor_add(x2, pv[:, :, 1, :],
                                             pw[:, :, 1, :])
                    ct = c_sb[:, i:i + 1, :].to_broadcast((128, 4, 64))
                    st = s_sb[:, i:i + 1, :].to_broadcast((128, 4, 64))
                    t1 = pat.tile([128, 4, 64], BF, name="t1", tag="t1")
                    nc.vector.tensor_mul(t1, x1, ct)
                    t2 = pat.tile([128, 4, 64], BF, name="t2", tag="t2")
                    nc.vector.tensor_mul(t2, x2, st)
                    t3 = pat.tile([128, 4, 64], BF, name="t3", tag="t3")
                    nc.vector.tensor_mul(t3, x2, ct)
                    t4 = pat.tile([128, 4, 64], BF, name="t4", tag="t4")
                    nc.vector.tensor_mul(t4, x1, st)
                    ro = pao.tile([128, 512], BF, name="ro", tag="ro")
                    rv = ro.rearrange("p (hh t z) -> p hh t z", hh=4, t=2, z=64)
                    nc.vector.tensor_sub(rv[:, :, 0, :], t1, t2)
                    nc.vector.tensor_add(rv[:, :, 1, :], t3, t4)
                    dst = qkrot[0 if kind == "q" else 1][grp]
                    nc.sync.dma_start(out=dst[ls, :], in_=ro)

                def emit_A_tile(c, i):
                    emit_A_finish(c, i, emit_A_alpha(c, i))

                def emit_A_dmajor(c, tiles):
                    # d-major across several open psum groups: every arriving
                    # x/w piece-group unlocks one alpha+2 betas per open tile
                    wa, wb = wch[c]
                    pns = [P["psa"].tile([128, 512], F32, name="pnat",
                                         tag="pnat") for _ in tiles]
                    for d2 in range(DT // 2):
                        for t, i in enumerate(tiles):
                            ls = slice(i * 128, (i + 1) * 128)
                            nc.tensor.matmul(
                                pns[t],
                                xall[:, 2 * d2:2 * d2 + 2, 1, ls],
                                wa[:, 2 * d2:2 * d2 + 2, :],
                                start=(d2 == 0), stop=False, perf_mode=DR)
                        for dd in (2 * d2, 2 * d2 + 1):
                            for t, i in enumerate(tiles):
                                ls = slice(i * 128, (i + 1) * 128)
                                nc.tensor.matmul(
                                    pns[t],
                                    xall[:, dd, :, ls],
                                    wb[:, dd, :, :],
                                    start=False, stop=(dd == DT - 1),
                                    perf_mode=DR)
                    for t, i in enumerate(tiles):
                        emit_A_rope(c, i, pns[t])

                # ---- emission: A(g0), then heads 0-3 x A(g1) ----
                if doA:
                    load_x()
                    # chunk 0 lead-in: 7 psum banks, d-major
                    with tc.tile_pool(name="psA0", bufs=8,
                                      space="PSUM") as psa0:
                        P["psa"] = psa0
                        emit_A_dmajor(0, range(8))
                        for i in range(8, LT):
                            emit_A_tile(0, i)
                        for c in (1, 2):
                            for i in range(LT):
                                emit_A_tile(c, i)
                with tc.tile_pool(name="psS", bufs=scb, space="PSUM") as pss, \
                     tc.tile_pool(name="psY", bufs=ypb, space="PSUM") as psy, \
                     tc.tile_pool(name="psD", bufs=1, space="PSUM") as psd:
                    P["pss"], P["psy"], P["psd"] = pss, psy, psd
                    with tc.tile_pool(name="psA", bufs=psab,
                                      space="PSUM") as psa:
                        P["psa"] = psa
                        g1 = [(c, i) for c in (3, 4, 5) for i in range(LT)] \
                            if doA else []
                        gi = 0
                        qts.clear()
                        slot = 0
                        for h in range(3 if doB else 0):
                            if h not in qts:
                                qts[h] = load_qk(h)
                            qt, kt = qts.pop(h)
                            for qc in range(QC):
                                emit_B_qc(h, qt, kt, qc)
                                if qc == 0 and h < 3 and doB:
                                    qts[h + 1] = load_qk(h + 1)
                                tk = takes[slot] if takes else take
                                slot += 1
                                for _ in range(tk):
                                    if gi < len(g1):
                                        emit_A_tile(*g1[gi])
                                        gi += 1
                        while gi < len(g1):
                            emit_A_tile(*g1[gi])
                            gi += 1

            # ---------------- heads 4-7 (qc-major) + phase C ----------
            with tc.tile_pool(name="pBy47", bufs=1) as pby47, \
                 tc.tile_pool(name="pCo", bufs=4) as pco, \
                 tc.tile_pool(name="psS3", bufs=scb, space="PSUM") as pss3, \
                 tc.tile_pool(name="psY3", bufs=ypb, space="PSUM") as psy3, \
                 tc.tile_pool(name="psD3", bufs=1, space="PSUM") as psd3, \
                 tc.tile_pool(name="psC", bufs=2, space="PSUM") as psc:
                P["pss"], P["psy"], P["psd"] = pss3, psy3, psd3
                qk47 = dict(qts)   # h3 was prefetched into the pb pool
                for h in range(4, NH if doB else 4):
                    grp, hh = h // 4, h % 4
                    qt = pby47.tile([128, L], BF, name=f"qt{h}", tag=f"qt{h}")
                    kt = pby47.tile([128, L], BF, name=f"kt{h}", tag=f"kt{h}")
                    for t, src in ((qt, qkrot[0][grp]), (kt, qkrot[1][grp])):
                        nc.sync.dma_start_transpose(
                            out=t, in_=src[:, hh * 128:(hh + 1) * 128])
                    qk47[h] = (qt, kt)
                yall[1] = pby47.tile([128, 4, 2, L], F8, name="yall1",
                                     tag="yall1")
                if not doB:
                    nc.vector.memset(yall[1], 0.0)
                woat = pby47.tile([128, NH, L], F8, name="woat", tag="woat")
                nc.sync.dma_start(
                    out=woat, in_=woA.rearrange("(dd p) e -> p dd e", p=128))
                wobt = pby47.tile([128, NH, 2, L], F8, name="wobt", tag="wobt")
                nc.sync.dma_start(
                    out=wobt.rearrange("p dd t e -> p dd (t e)"),
                    in_=woB.rearrange("(dd p) e -> p dd e", p=128))

                def emit_C(e, qc):
                    esl = slice(e * 128, (e + 1) * 128)
                    qsl = slice(qc * 512, (qc + 1) * 512)
                    op = psc.tile([128, 512], F32, name="op", tag="op")
                    for g in range(2):
                        for p2 in range(2):
                            dd = 4 * g + 2 * p2
                            nc.tensor.matmul(
                                op, woat[:, dd:dd + 2, esl],
                                yall[g][:, 2 * p2:2 * p2 + 2, 1, qsl],
                                start=(g == 0 and p2 == 0), stop=False,
                                perf_mode=DR)
                    for g in range(2):
                        for hh in range(4):
                            nc.tensor.matmul(
                                op, wobt[:, 4 * g + hh, :, esl],
                                yall[g][:, hh, :, qsl],
                                start=False, stop=(g == 1 and hh == 3),
                                perf_mode=DR)
                    ot = pco.tile([128, 512], BF, name="ot", tag="ot")
                    if (e + qc) % 2 == 0:
                        nc.scalar.activation(
                            out=ot, in_=op,
                            func=mybir.ActivationFunctionType.Copy,
                            scale=1.0 / 1024.0)
                    else:
                        nc.vector.tensor_scalar_mul(ot, op, 1.0 / 1024.0)
                    nc.sync.dma_start(
                        out=outT[e * 128:(e + 1) * 128,
                                 qc * 512:(qc + 1) * 512],
                        in_=ot)

                for qc in range(QC if doB else 0):
                    for h in range(3, NH):
                        emit_B_qc(h, qk47[h][0], qk47[h][1], qc)
                    if doC and qc > 0:
                        for e in range(DT):
                            emit_C(e, qc - 1)
                if doC:
                    for qc in ([3] if doB else range(QC)):
                        for e in range(DT):
                            emit_C(e, qc)
    nc.compile()
    return nc


_NC_CACHE = None


def _get_program():
    global _NC_CACHE
    if _NC_CACHE is None:
        _NC_CACHE = build_program()
    return _NC_CACHE


def _f8(a):
    return np.clip(np.asarray(a, np.float64), -240.0, 240.0).astype(F8NP)


# within each head's 128 comps: evens first, then odds
_PERM512 = np.concatenate(
    [np.concatenate([np.arange(h * 128, (h + 1) * 128, 2),
                     np.arange(h * 128 + 1, (h + 1) * 128, 2)])
     for h in range(4)])


def _host_inputs(x, w_qkv, w_o):
    inv = 1.0 / (ROPE_BASE ** (np.arange(0, HD, 2, dtype=np.float64) / HD))
    ang = np.arange(L, dtype=np.float64)[:, None] * inv[None, :]
    chalf = (np.cos(ang) / 1024.0).astype(BF16)                # [L, 64]
    shalf = (np.sin(ang) / 1024.0).astype(BF16)
    p = np.arange(128)[:, None]
    f = np.arange(128)[None, :]
    tri = (p <= f).astype(np.float16)                          # [128, 128]

    xhi_all, xlo_all = [], []
    for b in range(B):
        xb = np.asarray(x[b], np.float64)                      # [L, D]
        xh = _f8(xb)
        xl = _f8(16.0 * (xb - xh.astype(np.float64)))
        xhi_all.append(np.ascontiguousarray(xh.T))
        xlo_all.append(np.ascontiguousarray(xl.T))

    in_maps = []
    for c in range(8):
        b, g = c % 4, c // 4
        qr = w_qkv[g * DL:(g + 1) * DL]
        kr = w_qkv[D + g * DL:D + (g + 1) * DL]
        vr = w_qkv[2 * D + g * DL:2 * D + (g + 1) * DL]
        qp = qr[np.concatenate([_PERM512, 512 + _PERM512])]
        kp = kr[np.concatenate([_PERM512, 512 + _PERM512])]
        wloc = np.concatenate([qp[:512], kp[:512], vr[:512],
                               qp[512:], kp[512:], vr[512:]],
                              axis=0).astype(np.float64)       # [3DL, D]
        w64h = _f8(64.0 * wloc)
        w1024h = (w64h.astype(np.float64) * 16.0).astype(F8NP)  # exact shift
        w16l = _f8(16.0 * (64.0 * wloc - w64h.astype(np.float64)))
        wAh = np.ascontiguousarray(w1024h.T)                    # [D, 3DL]
        w64hT, w16lT = w64h.T, w16l.T                           # [D, 3DL]
        wBh = np.ascontiguousarray(np.concatenate(
            [np.stack([w64hT[:, c2 * 512:(c2 + 1) * 512],
                       w16lT[:, c2 * 512:(c2 + 1) * 512]],
                      axis=1).reshape(D, 1024)
             for c2 in range(NCH)], axis=1))                    # [D, 6DL]
        woT64 = 64.0 * w_o[:, g * DL:(g + 1) * DL].T.astype(np.float64)
        wo64h = _f8(woT64)
        woAh = np.ascontiguousarray(
            (wo64h.astype(np.float64) * 16.0).astype(F8NP))      # [DL, L]
        wo16l = _f8(16.0 * (woT64 - wo64h.astype(np.float64)))
        woBh = np.ascontiguousarray(
            np.concatenate([wo64h[:, None, :], wo16l[:, None, :]],
                           axis=1).reshape(DL, 2 * L))
        in_maps.append({
            "xThi": xhi_all[b],
            "xTlo": xlo_all[b],
            "wA": wAh,
            "wB": wBh,
            "woA": woAh,
            "woB": woBh,
            "chalf": chalf,
            "shalf": shalf,
            "tri": tri,
        })
    return in_maps


def kernel(x, w_qkv, w_o, _trace=False):
    x = np.asarray(x, dtype=np.float32)
    w_qkv = np.asarray(w_qkv, dtype=np.float32)
    w_o = np.asarray(w_o, dtype=np.float32)
    nc = _get_program()
    in_maps = _host_inputs(x, w_qkv, w_o)
    res = run_bass_kernel_spmd(nc, in_maps, core_ids=list(range(8)),
                               trace=_trace)
    kernel.last_result = res
    parts = [r["outT"].astype(np.float32) for r in res.results]
    out = np.empty((B, L, D), dtype=np.float32)
    for b in range(B):
        out[b] = (parts[b] + parts[b + 4]).T
    return out


# revision 10
# speedup vs baseline: 1.0127x; 1.0067x over previous
"""MHA (RoPE + causal softmax attention + out-proj) on 8 NeuronCores — v3.

Sharding: DP4 x TP2 (core c: batch c % 4, head-group c // 4; 8 heads/core).
Host sums the two head-group partial outputs per batch and transposes.

Key structure (tuned against the TimelineSim cost model):
  * Phase A (QKV) matmuls run in fp8 e4m3 DoubleRow with a hi/lo split of
    both operands (x ~ xh + xl/16, 64w ~ wh + wl/16, lo*lo dropped):
    3 DR instructions per d-tile pair = 0.75x the bf16 instruction cost at
    better-than-bf16 accuracy. PSUM carries 1024*qkv; the 2^-10 unscale is
    folded into the RoPE-cast / v copy scales.
  * q/k comps are host-permuted to (evens | odds) within each head so RoPE
    reads contiguous PSUM blocks: 2 ACT casts + 6 DVE bf16 ops (4x mode).
  * Softmax: exp(alpha*s - 8ln2) -> fp16 exp tiles; denominator accumulated
    with DVE adds + ONE ones-matmul per (head, qc) instead of a ones-matmul
    per k-tile. Causal diagonal tiles are column-trimmed; a single [128,128]
    triangle mask remains.
  * v never round-trips DRAM (PSUM -> SBUF fp16 copy, resident).
  * Emission interleaves head-group-1 QKV tiles into the attention loop of
    head-group-0 so the exp-bound stretch of attention overlaps the
    PE-bound QKV GEMM instead of stalling the tensor engine.
"""

import numpy as np
import ml_dtypes

import concourse.bass as bass
import concourse.tile as tile
import concourse.mybir as mybir
from concourse import bacc
from concourse.bass_utils import run_bass_kernel_spmd

BF16 = ml_dtypes.bfloat16
F8NP = ml_dtypes.float8_e4m3
F32 = mybir.dt.float32
BF = mybir.dt.bfloat16
F16 = mybir.dt.float16
F8 = mybir.dt.float8e4
DR = mybir.MatmulPerfMode.DoubleRow

B, L, D, H, HD = 4, 2048, 2048, 16, 128
NH = 8                      # heads per core
DL = NH * HD                # 1024 local head dims
ROPE_BASE = 10000.0
ALPHA = float(HD) ** -0.5
EXP_BIAS = -8.0 * float(np.log(2.0))   # exp(a*s - 8ln2): keeps fp16 sums safe

LT = L // 128               # 16 L-tiles
DT = D // 128               # 16 D(contract)-tiles
NCH = 6                     # qkv chunks of 512 comps: q03,k03,v03,q47,k47,v47
QC = L // 512               # 4 q-chunks of 512
KT = L // 128               # 16 k-tiles


def _chunk_kind(c):
    # chunk order: q(heads0-3), k(0-3), v(0-3), q(4-7), k(4-7), v(4-7)
    return ("q", "k", "v")[c % 3], c // 3


def build_program(phases="ABC", la=3, scb=3, ypb=1, psab=3, patb=1, paob=3,
                  pbab=5, take=4, takes=None):
    nc = bacc.Bacc("TRN2", target_bir_lowering=False, debug=False, num_devices=8)

    # x hi/lo fp8 planes: hi = fp8(x), lo = fp8(16*(x-hi))
    xThi = nc.dram_tensor("xThi", [D, L], F8, kind="ExternalInput").ap()
    xTlo = nc.dram_tensor("xTlo", [D, L], F8, kind="ExternalInput").ap()
    # w planes: wA = fp8(16*w64_hi) [D, 3DL]; wB [D, 2*3DL]: per 512-chunk,
    # 1024 cols = (w64_hi 512 | w16_lo 512)
    wA = nc.dram_tensor("wA", [D, 3 * DL], F8, kind="ExternalInput").ap()
    wB = nc.dram_tensor("wB", [D, 6 * DL], F8, kind="ExternalInput").ap()
    woA = nc.dram_tensor("woA", [DL, L], F8, kind="ExternalInput").ap()
    woB = nc.dram_tensor("woB", [DL, 2 * L], F8, kind="ExternalInput").ap()
    chalf = nc.dram_tensor("chalf", [L, 64], BF, kind="ExternalInput").ap()
    shalf = nc.dram_tensor("shalf", [L, 64], BF, kind="ExternalInput").ap()
    tri = nc.dram_tensor("tri", [128, 128], F16, kind="ExternalInput").ap()
    outT = nc.dram_tensor("outT", [D, L], BF, kind="ExternalOutput").ap()

    # DRAM staging for rotated q/k, split per head-group so group-0 attention
    # does not depend on group-1 writes
    qkrot = [[nc.dram_tensor(f"{nm}rot{g}", [L, 512], BF, kind="Internal").ap()
              for g in range(2)] for nm in ("q", "k")]

    doA = "A" in phases
    doB = "B" in phases
    doC = "C" in phases

    with tile.TileContext(nc) as tc:
        outer_cm = tc.tile_pool(name="outer", bufs=1)
        pb_cm = tc.tile_pool(name="pBqk", bufs=2, side="right")
        pbm_cm = tc.tile_pool(name="pBm", bufs=1, side="right")
        pby03_cm = tc.tile_pool(name="pBy03", bufs=1)
        pba_cm = tc.tile_pool(name="pBa", bufs=pbab)
        pbr_cm = tc.tile_pool(name="pBr", bufs=1)
        pbv_cm = tc.tile_pool(name="pBv", bufs=1)
        pbd_cm = tc.tile_pool(name="pBd", bufs=2)
        P = {}
        with outer_cm as outer, pb_cm as pb, pbm_cm as pbm, \
             pby03_cm as pby03, pba_cm as pba, pbr_cm as pbr, \
             pbv_cm as pbv, pbd_cm as pbd:
            # v for both head groups, resident SBUF: [128(kpos), LT, 512]
            vsb = [outer.tile([128, LT, 512], F16, name=f"vsb{g}",
                              tag=f"vsb{g}") for g in range(2)]
            ones128 = outer.tile([128, 128], F16, name="ones128", tag="oc")
            nc.vector.memset(ones128, 1.0)
            ebias = outer.tile([128, 1], F32, name="ebias", tag="ebias")
            nc.vector.memset(ebias, EXP_BIAS)
            trit = pbm.tile([128, 128], F16, name="tri", tag="tri")
            nc.sync.dma_start(out=trit, in_=tri)

            qts = {}
            yall = {}
            yall[0] = pby03.tile([128, 4, 2, L], F8, name="yall0", tag="yall0")
            if not doB:
                nc.vector.memset(yall[0], 0.0)

            # ---------------- phase B helpers ----------------
            def load_qk(h):
                grp, hh = h // 4, h % 4
                qt = pb.tile([128, L], BF, name="qt", tag="qt")
                kt = pb.tile([128, L], BF, name="kt", tag="kt")
                for t, src in ((qt, qkrot[0][grp]), (kt, qkrot[1][grp])):
                    nc.sync.dma_start_transpose(
                        out=t, in_=src[:, hh * 128:(hh + 1) * 128])
                return qt, kt

            def emit_B_qc(h, qt, kt, qc):
                grp, hh = h // 4, h % 4
                nkt = 4 * qc + 4
                ypsum = P["psy"].tile([128, 512], F32, name="ypsum", tag="yp")
                dacc = pbd.tile([128, 512], F16, name="dacc", tag="dacc")
                ats = {}

                def emit_score(j):
                    m = j - 4 * qc
                    off = 128 * m if m > 0 else 0
                    w = 512 - off
                    sc = P["pss"].tile([128, 512], F32, name="sc", tag="sc")
                    nc.tensor.matmul(
                        sc[:, 0:w], kt[:, j * 128:(j + 1) * 128],
                        qt[:, qc * 512 + off:(qc + 1) * 512],
                        start=True, stop=True)
                    at = pba.tile([128, 512], F16, name="at", tag="at")
                    nc.scalar.activation(
                        out=at[:, 0:w], in_=sc[:, 0:w],
                        func=mybir.ActivationFunctionType.Exp,
                        scale=ALPHA, bias=ebias)
                    if m >= 0:
                        nc.vector.tensor_mul(at[:, 0:128], at[:, 0:128], trit)
                    ats[j] = (at, off, w)

                for j in range(min(la, nkt)):
                    emit_score(j)
                prev = None
                for j in range(nkt):
                    if j + la < nkt:
                        emit_score(j + la)
                    at, off, w = ats.pop(j)
                    nc.tensor.matmul(
                        ypsum[:, off:512],
                        vsb[grp][:, j, hh * 128:(hh + 1) * 128],
                        at[:, 0:w],
                        start=(j == 0), stop=(j == nkt - 1),
                        skip_group_check=True)
                    if j == 0:
                        prev = at
                    elif j == 1:
                        if qc == 0:
                            nc.vector.tensor_copy(
                                out=dacc[:, 0:128], in_=prev[:, 0:128])
                            nc.vector.tensor_add(
                                dacc[:, 128:512], prev[:, 128:512],
                                at[:, 0:w])
                        else:
                            nc.vector.tensor_add(dacc, prev, at)
                    else:
                        nc.vector.tensor_add(
                            dacc[:, off:512], dacc[:, off:512], at[:, 0:w])
                dpsum = P["psd"].tile([128, 512], F32, name="dpsum", tag="dp")
                nc.tensor.matmul(dpsum, ones128, dacc, start=True, stop=True)
                rbs = pbr.tile([128, 512], BF, name="rbs", tag="rbs")
                with nc.allow_low_precision("softmax recip bf16"):
                    nc.vector.reciprocal(out=rbs, in_=dpsum)
                qsl = slice(qc * 512, (qc + 1) * 512)
                ya = yall[grp]
                yb = pbv.tile([128, 512], BF, name="yb", tag="yb")
                nc.vector.tensor_mul(yb, ypsum, rbs)
                nc.gpsimd.tensor_copy(out=ya[:, hh, 1, qsl], in_=yb)
                yd = pbv.tile([128, 512], BF, name="yd", tag="yd")
                nc.gpsimd.tensor_sub(yd, yb, ya[:, hh, 1, qsl])
                nc.gpsimd.tensor_scalar_mul(ya[:, hh, 0, qsl], yd, 16.0)

            # ---------------- phase A scope + interleave ----------------
            with tc.tile_pool(name="pA", bufs=1) as pa, \
                 tc.tile_pool(name="pAw", bufs=2) as paw, \
                 tc.tile_pool(name="pAt", bufs=patb) as pat, \
                 tc.tile_pool(name="pAp", bufs=16) as pap, \
                 tc.tile_pool(name="pAo", bufs=paob) as pao:
                xall = pa.tile([128, DT, 2, L], F8, name="xall", tag="xall")
                c_sb = pa.tile([128, LT, 64], BF, name="c_sb", tag="c_sb")
                s_sb = pa.tile([128, LT, 64], BF, name="s_sb", tag="s_sb")
                wch = {}

                def load_wch(c):
                    if c >= NCH or c in wch:
                        return
                    wa = paw.tile([128, DT, 512], F8, name="wchA", tag="wchA")
                    wb = paw.tile([128, DT, 2, 512], F8, name="wchB",
                                  tag="wchB")
                    wAr = wA[:, c * 512:(c + 1) * 512].rearrange(
                        "(d p) e -> p d e", p=128)
                    wBr = wB[:, c * 1024:(c + 1) * 1024].rearrange(
                        "(d p) e -> p d e", p=128)
                    wbf = wb.rearrange("p d t e -> p d (t e)")
                    for d4 in range(DT // 4):
                        sl = slice(4 * d4, 4 * d4 + 4)
                        nc.sync.dma_start(out=wa[:, sl, :], in_=wAr[:, sl, :])
                        nc.sync.dma_start(out=wbf[:, sl, :], in_=wBr[:, sl, :])
                    wch[c] = (wa, wb)

                def load_x():
                    wa = paw.tile([128, DT, 512], F8, name="wchA", tag="wchA")
                    wb = paw.tile([128, DT, 2, 512], F8, name="wchB",
                                  tag="wchB")
                    wAr = wA[:, 0:512].rearrange("(d p) e -> p d e", p=128)
                    wBr = wB[:, 0:1024].rearrange("(d p) e -> p d e", p=128)
                    wbf = wb.rearrange("p d t e -> p d (t e)")
                    xhr = xThi.rearrange("(d p) l -> p d l", p=128)
                    xlr = xTlo.rearrange("(d p) l -> p d l", p=128)
                    nc.sync.dma_start(out=xall[:, 0:2, 1, :],
                                      in_=xhr[:, 0:2, :])
                    nc.sync.dma_start(out=wa[:, 0:4, :], in_=wAr[:, 0:4, :])
                    for d2 in range(DT // 2):
                        if d2 == 0:
                            nc.sync.dma_start(out=xall[:, 0:2, 0, :],
                                              in_=xlr[:, 0:2, :])
                            continue
                        sl = slice(2 * d2, 2 * d2 + 2)
                        nc.sync.dma_start(out=xall[:, sl, 1, :],
                                          in_=xhr[:, sl, :])
                        nc.sync.dma_start(out=xall[:, sl, 0, :],
                                          in_=xlr[:, sl, :])
                        if d2 % 2 == 1:
                            sl4 = slice(2 * d2 - 2, 2 * d2 + 2)
                            if d2 > 1:
                                nc.sync.dma_start(out=wa[:, sl4, :],
                                                  in_=wAr[:, sl4, :])
                            nc.sync.dma_start(out=wbf[:, sl4, :],
                                              in_=wBr[:, sl4, :])
                            if d2 == 1:
                                for t_sb, t_dr in ((c_sb, chalf),
                                                   (s_sb, shalf)):
                                    nc.sync.dma_start(
                                        out=t_sb,
                                        in_=t_dr.rearrange(
                                            "(i p) g -> p i g", p=128))
                    wch[0] = (wa, wb)

                def emit_A_alpha(c, i):
                    wa, _ = wch[c]
                    ls = slice(i * 128, (i + 1) * 128)
                    pnat = P["psa"].tile([128, 512], F32, name="pnat",
                                         tag="pnat")
                    for d2 in range(DT // 2):
                        nc.tensor.matmul(
                            pnat,
                            xall[:, 2 * d2:2 * d2 + 2, 1, ls],
                            wa[:, 2 * d2:2 * d2 + 2, :],
                            start=(d2 == 0), stop=False, perf_mode=DR)
                    return pnat

                def emit_A_finish(c, i, pnat):
                    _, wb = wch[c]
                    if i == 8:
                        load_wch(c + 1)
                    ls = slice(i * 128, (i + 1) * 128)
                    for d in range(DT):
                        nc.tensor.matmul(
                            pnat,
                            xall[:, d, :, ls],
                            wb[:, d, :, :],
                            start=False, stop=(d == DT - 1), perf_mode=DR)
                    emit_A_rope(c, i, pnat)

                def emit_A_rope(c, i, pnat, part=None):
                    kind, grp = _chunk_kind(c)
                    ls = slice(i * 128, (i + 1) * 128)
                    if kind == "v":
                        nc.scalar.activation(
                            out=vsb[grp][:, i, :], in_=pnat,
                            func=mybir.ActivationFunctionType.Copy,
                            scale=1.0 / 1024.0)
                        return
                    # RoPE: per-head comps are permuted (evens | odds)
                    pv = pnat.rearrange("p (hh t z) -> p hh t z",
                                        hh=4, t=2, z=64)
                    x1 = pat.tile([128, 4, 64], BF, name="x1", tag="x1")
                    x2 = pat.tile([128, 4, 64], BF, name="x2", tag="x2")
                    if part is None:
                        nc.scalar.activation(
                            out=x1, in_=pv[:, :, 0, :],
                            func=mybir.ActivationFunctionType.Copy)
                        nc.scalar.activation(
                            out=x2, in_=pv[:, :, 1, :],
                            func=mybir.ActivationFunctionType.Copy)
                    else:
                        pw = part.rearrange("p (hh t z) -> p hh t z",
                                            hh=4, t=2, z=64)
                        nc.vector.tensor_add(x1, pv[:, :, 0, :],
                                             pw[:, :, 0, :])
                        nc.vector.tensor_add(x2, pv[:, :, 1, :],
                                             pw[:, :, 1, :])
                    ct = c_sb[:, i:i + 1, :].to_broadcast((128, 4, 64))
                    st = s_sb[:, i:i + 1, :].to_broadcast((128, 4, 64))
                    t1 = pat.tile([128, 4, 64], BF, name="t1", tag="t1")
                    nc.vector.tensor_mul(t1, x1, ct)
                    t2 = pat.tile([128, 4, 64], BF, name="t2", tag="t2")
                    nc.vector.tensor_mul(t2, x2, st)
                    t3 = pat.tile([128, 4, 64], BF, name="t3", tag="t3")
                    nc.vector.tensor_mul(t3, x2, ct)
                    t4 = pat.tile([128, 4, 64], BF, name="t4", tag="t4")
                    nc.vector.tensor_mul(t4, x1, st)
                    ro = pao.tile([128, 512], BF, name="ro", tag="ro")
                    rv = ro.rearrange("p (hh t z) -> p hh t z", hh=4, t=2, z=64)
                    nc.vector.tensor_sub(rv[:, :, 0, :], t1, t2)
                    nc.vector.tensor_add(rv[:, :, 1, :], t3, t4)
                    dst = qkrot[0 if kind == "q" else 1][grp]
                    nc.sync.dma_start(out=dst[ls, :], in_=ro)

                def emit_A_tile(c, i):
                    emit_A_finish(c, i, emit_A_alpha(c, i))

                def emit_A_dmajor(c, tiles):
                    # d-major across several open psum groups: every arriving
                    # x/w piece-group unlocks one alpha+2 betas per open tile
                    wa, wb = wch[c]
                    pns = [P["psa"].tile([128, 512], F32, name="pnat",
                                         tag="pnat") for _ in tiles]
                    for d2 in range(DT // 2):
                        for t, i in enumerate(tiles):
                            ls = slice(i * 128, (i + 1) * 128)
                            nc.tensor.matmul(
                                pns[t],
                                xall[:, 2 * d2:2 * d2 + 2, 1, ls],
                                wa[:, 2 * d2:2 * d2 + 2, :],
                                start=(d2 == 0), stop=False, perf_mode=DR)
                        for dd in (2 * d2, 2 * d2 + 1):
                            for t, i in enumerate(tiles):
                                ls = slice(i * 128, (i + 1) * 128)
                                nc.tensor.matmul(
                                    pns[t],
                                    xall[:, dd, :, ls],
                                    wb[:, dd, :, :],
                                    start=False, stop=(dd == DT - 1),
                                    perf_mode=DR)
                    for t, i in enumerate(tiles):
                        emit_A_rope(c, i, pns[t])

                # ---- emission: A(g0), then heads 0-3 x A(g1) ----
                if doA:
                    load_x()
                    # chunk 0 lead-in: 7 psum banks, d-major
                    with tc.tile_pool(name="psA0", bufs=8,
                                      space="PSUM") as psa0:
                        P["psa"] = psa0
                        emit_A_dmajor(0, range(8))
                        for i in range(8, LT):
                            emit_A_tile(0, i)
                        for c in (1, 2):
                            for i in range(LT):
                                emit_A_tile(c, i)
                with tc.tile_pool(name="psS", bufs=scb, space="PSUM") as pss, \
                     tc.tile_pool(name="psY", bufs=ypb, space="PSUM") as psy, \
                     tc.tile_pool(name="psD", bufs=1, space="PSUM") as psd:
                    P["pss"], P["psy"], P["psd"] = pss, psy, psd
                    with tc.tile_pool(name="psA", bufs=psab,
                                      space="PSUM") as psa:
                        P["psa"] = psa
                        g1 = [(c, i) for c in (3, 4, 5) for i in range(LT)] \
                            if doA else []
                        gi = 0
                        qts.clear()
                        slot = 0
                        for h in range(3 if doB else 0):
                            if h not in qts:
                                qts[h] = load_qk(h)
                            qt, kt = qts.pop(h)
                            for qc in range(QC):
                                emit_B_qc(h, qt, kt, qc)
                                if qc == 0 and h < 3 and doB:
                                    qts[h + 1] = load_qk(h + 1)
                                tk = takes[slot] if takes else take
                                slot += 1
                                for _ in range(tk):
                                    if gi < len(g1):
                                        emit_A_tile(*g1[gi])
                                        gi += 1
                        while gi < len(g1):
                            emit_A_tile(*g1[gi])
                            gi += 1

            # ---------------- heads 4-7 (qc-major) + phase C ----------
            with tc.tile_pool(name="pBy47", bufs=1) as pby47, \
                 tc.tile_pool(name="pCo", bufs=4) as pco, \
                 tc.tile_pool(name="psS3", bufs=scb, space="PSUM") as pss3, \
                 tc.tile_pool(name="psY3", bufs=ypb, space="PSUM") as psy3, \
                 tc.tile_pool(name="psD3", bufs=1, space="PSUM") as psd3, \
                 tc.tile_pool(name="psC", bufs=2, space="PSUM") as psc:
                P["pss"], P["psy"], P["psd"] = pss3, psy3, psd3
                qk47 = dict(qts)   # h3 was prefetched into the pb pool
                for h in range(4, NH if doB else 4):
                    grp, hh = h // 4, h % 4
                    qt = pby47.tile([128, L], BF, name=f"qt{h}", tag=f"qt{h}")
                    kt = pby47.tile([128, L], BF, name=f"kt{h}", tag=f"kt{h}")
                    for t, src in ((qt, qkrot[0][grp]), (kt, qkrot[1][grp])):
                        nc.sync.dma_start_transpose(
                            out=t, in_=src[:, hh * 128:(hh + 1) * 128])
                    qk47[h] = (qt, kt)
                yall[1] = pby47.tile([128, 4, 2, L], F8, name="yall1",
                                     tag="yall1")
                if not doB:
                    nc.vector.memset(yall[1], 0.0)
                woat = pby47.tile([128, NH, L], F8, name="woat", tag="woat")
                nc.sync.dma_start(
                    out=woat, in_=woA.rearrange("(dd p) e -> p dd e", p=128))
                wobt = pby47.tile([128, NH, 2, L], F8, name="wobt", tag="wobt")
                nc.sync.dma_start(
                    out=wobt.rearrange("p dd t e -> p dd (t e)"),
                    in_=woB.rearrange("(dd p) e -> p dd e", p=128))

                def emit_C(e, qc):
                    esl = slice(e * 128, (e + 1) * 128)
                    qsl = slice(qc * 512, (qc + 1) * 512)
                    op = psc.tile([128, 512], F32, name="op", tag="op")
                    for g in range(2):
                        for p2 in range(2):
                            dd = 4 * g + 2 * p2
                            nc.tensor.matmul(
                                op, woat[:, dd:dd + 2, esl],
                                yall[g][:, 2 * p2:2 * p2 + 2, 1, qsl],
                                start=(g == 0 and p2 == 0), stop=False,
                                perf_mode=DR)
                    for g in range(2):
                        for hh in range(4):
                            nc.tensor.matmul(
                                op, wobt[:, 4 * g + hh, :, esl],
                                yall[g][:, hh, :, qsl],
                                start=False, stop=(g == 1 and hh == 3),
                                perf_mode=DR)
                    ot = pco.tile([128, 512], BF, name="ot", tag="ot")
                    if (e + qc) % 2 == 0:
                        nc.scalar.activation(
                            out=ot, in_=op,
                            func=mybir.ActivationFunctionType.Copy,
                            scale=1.0 / 1024.0)
                    else:
                        nc.vector.tensor_scalar_mul(ot, op, 1.0 / 1024.0)
                    nc.sync.dma_start(
                        out=outT[e * 128:(e + 1) * 128,
                                 qc * 512:(qc + 1) * 512],
                        in_=ot)

                for qc in range(QC if doB else 0):
                    for h in range(3, NH):
                        emit_B_qc(h, qk47[h][0], qk47[h][1], qc)
                    if doC and qc > 0:
                        for e in range(DT):
                            emit_C(e, qc - 1)
                if doC:
                    for qc in ([3] if doB else range(QC)):
                        for e in range(DT):
                            emit_C(e, qc)
    nc.compile()
    return nc


_NC_CACHE = None


def _get_program():
    global _NC_CACHE
    if _NC_CACHE is None:
        _NC_CACHE = build_program()
    return _NC_CACHE


def _f8(a):
    return np.clip(np.asarray(a, np.float64), -240.0, 240.0).astype(F8NP)


# within each head's 128 comps: evens first, then odds
_PERM512 = np.concatenate(
    [np.concatenate([np.arange(h * 128, (h + 1) * 128, 2),
                     np.arange(h * 128 + 1, (h + 1) * 128, 2)])
     for h in range(4)])


def _host_inputs(x, w_qkv, w_o):
    inv = 1.0 / (ROPE_BASE ** (np.arange(0, HD, 2, dtype=np.float64) / HD))
    ang = np.arange(L, dtype=np.float64)[:, None] * inv[None, :]
    chalf = (np.cos(ang) / 1024.0).astype(BF16)                # [L, 64]
    shalf = (np.sin(ang) / 1024.0).astype(BF16)
    p = np.arange(128)[:, None]
    f = np.arange(128)[None, :]
    tri = (p <= f).astype(np.float16)                          # [128, 128]

    xhi_all, xlo_all = [], []
    for b in range(B):
        xb = np.asarray(x[b], np.float64)                      # [L, D]
        xh = _f8(xb)
        xl = _f8(16.0 * (xb - xh.astype(np.float64)))
        xhi_all.append(np.ascontiguousarray(xh.T))
        xlo_all.append(np.ascontiguousarray(xl.T))

    in_maps = []
    for c in range(8):
        b, g = c % 4, c // 4
        qr = w_qkv[g * DL:(g + 1) * DL]
        kr = w_qkv[D + g * DL:D + (g + 1) * DL]
        vr = w_qkv[2 * D + g * DL:2 * D + (g + 1) * DL]
        qp = qr[np.concatenate([_PERM512, 512 + _PERM512])]
        kp = kr[np.concatenate([_PERM512, 512 + _PERM512])]
        wloc = np.concatenate([qp[:512], kp[:512], vr[:512],
                               qp[512:], kp[512:], vr[512:]],
                              axis=0).astype(np.float64)       # [3DL, D]
        w64h = _f8(64.0 * wloc)
        w1024h = (w64h.astype(np.float64) * 16.0).astype(F8NP)  # exact shift
        w16l = _f8(16.0 * (64.0 * wloc - w64h.astype(np.float64)))
        wAh = np.ascontiguousarray(w1024h.T)                    # [D, 3DL]
        w64hT, w16lT = w64h.T, w16l.T                           # [D, 3DL]
        wBh = np.ascontiguousarray(np.concatenate(
            [np.stack([w64hT[:, c2 * 512:(c2 + 1) * 512],
                       w16lT[:, c2 * 512:(c2 + 1) * 512]],
                      axis=1).reshape(D, 1024)
             for c2 in range(NCH)], axis=1))                    # [D, 6DL]
        woT64 = 64.0 * w_o[:, g * DL:(g + 1) * DL].T.astype(np.float64)
        wo64h = _f8(woT64)
        woAh = np.ascontiguousarray(
            (wo64h.astype(np.float64) * 16.0).astype(F8NP))      # [DL, L]
        wo16l = _f8(16.0 * (woT64 - wo64h.astype(np.float64)))
        woBh = np.ascontiguousarray(
            np.concatenate([wo64h[:, None, :], wo16l[:, None, :]],
                           axis=1).reshape(DL, 2 * L))
        in_maps.append({
            "xThi": xhi_all[b],
            "xTlo": xlo_all[b],
            "wA": wAh,
            "wB": wBh,
            "woA": woAh,
            "woB": woBh,
            "chalf": chalf,
            "shalf": shalf,
            "tri": tri,
        })
    return in_maps


def kernel(x, w_qkv, w_o, _trace=False):
    x = np.asarray(x, dtype=np.float32)
    w_qkv = np.asarray(w_qkv, dtype=np.float32)
    w_o = np.asarray(w_o, dtype=np.float32)
    nc = _get_program()
    in_maps = _host_inputs(x, w_qkv, w_o)
    res = run_bass_kernel_spmd(nc, in_maps, core_ids=list(range(8)),
                               trace=_trace)
    kernel.last_result = res
    parts = [r["outT"].astype(np.float32) for r in res.results]
    out = np.empty((B, L, D), dtype=np.float32)
    for b in range(B):
        out[b] = (parts[b] + parts[b + 4]).T
    return out


# revision 11
# speedup vs baseline: 1.0170x; 1.0042x over previous
"""MHA (RoPE + causal softmax attention + out-proj) on 8 NeuronCores — v3.

Sharding: DP4 x TP2 (core c: batch c % 4, head-group c // 4; 8 heads/core).
Host sums the two head-group partial outputs per batch and transposes.

Key structure (tuned against the TimelineSim cost model):
  * Phase A (QKV) matmuls run in fp8 e4m3 DoubleRow with a hi/lo split of
    both operands (x ~ xh + xl/16, 64w ~ wh + wl/16, lo*lo dropped):
    3 DR instructions per d-tile pair = 0.75x the bf16 instruction cost at
    better-than-bf16 accuracy. PSUM carries 1024*qkv; the 2^-10 unscale is
    folded into the RoPE-cast / v copy scales.
  * q/k comps are host-permuted to (evens | odds) within each head so RoPE
    reads contiguous PSUM blocks: 2 ACT casts + 6 DVE bf16 ops (4x mode).
  * Softmax: exp(alpha*s - 8ln2) -> fp16 exp tiles; denominator accumulated
    with DVE adds + ONE ones-matmul per (head, qc) instead of a ones-matmul
    per k-tile. Causal diagonal tiles are column-trimmed; a single [128,128]
    triangle mask remains.
  * v never round-trips DRAM (PSUM -> SBUF fp16 copy, resident).
  * Emission interleaves head-group-1 QKV tiles into the attention loop of
    head-group-0 so the exp-bound stretch of attention overlaps the
    PE-bound QKV GEMM instead of stalling the tensor engine.
"""

import numpy as np
import ml_dtypes

import concourse.bass as bass
import concourse.tile as tile
import concourse.mybir as mybir
from concourse import bacc
from concourse.bass_utils import run_bass_kernel_spmd

BF16 = ml_dtypes.bfloat16
F8NP = ml_dtypes.float8_e4m3
F32 = mybir.dt.float32
BF = mybir.dt.bfloat16
F16 = mybir.dt.float16
F8 = mybir.dt.float8e4
DR = mybir.MatmulPerfMode.DoubleRow

B, L, D, H, HD = 4, 2048, 2048, 16, 128
NH = 8                      # heads per core
DL = NH * HD                # 1024 local head dims
ROPE_BASE = 10000.0
ALPHA = float(HD) ** -0.5
EXP_BIAS = -8.0 * float(np.log(2.0))   # exp(a*s - 8ln2): keeps fp16 sums safe

LT = L // 128               # 16 L-tiles
DT = D // 128               # 16 D(contract)-tiles
NCH = 6                     # qkv chunks of 512 comps: q03,k03,v03,q47,k47,v47
QC = L // 512               # 4 q-chunks of 512
KT = L // 128               # 16 k-tiles


def _chunk_kind(c):
    # chunk order: q(heads0-3), k(0-3), v(0-3), q(4-7), k(4-7), v(4-7)
    return ("q", "k", "v")[c % 3], c // 3


def build_program(phases="ABC", la=3, scb=3, ypb=1, psab=3, patb=1, paob=3,
                  pbab=5, take=4, takes=None):
    nc = bacc.Bacc("TRN2", target_bir_lowering=False, debug=False, num_devices=8)

    # x hi/lo fp8 planes: hi = fp8(x), lo = fp8(16*(x-hi))
    xThi = nc.dram_tensor("xThi", [D, L], F8, kind="ExternalInput").ap()
    xTlo = nc.dram_tensor("xTlo", [D, L], F8, kind="ExternalInput").ap()
    # w planes: wA = fp8(16*w64_hi) [D, 3DL]; wB [D, 2*3DL]: per 512-chunk,
    # 1024 cols = (w64_hi 512 | w16_lo 512)
    wA = nc.dram_tensor("wA", [D, 3 * DL], F8, kind="ExternalInput").ap()
    wB = nc.dram_tensor("wB", [D, 6 * DL], F8, kind="ExternalInput").ap()
    woA = nc.dram_tensor("woA", [DL, L], F8, kind="ExternalInput").ap()
    woB = nc.dram_tensor("woB", [DL, 2 * L], F8, kind="ExternalInput").ap()
    chalf = nc.dram_tensor("chalf", [L, 64], BF, kind="ExternalInput").ap()
    shalf = nc.dram_tensor("shalf", [L, 64], BF, kind="ExternalInput").ap()
    tri = nc.dram_tensor("tri", [128, 128], F16, kind="ExternalInput").ap()
    outT = nc.dram_tensor("outT", [D, L], BF, kind="ExternalOutput").ap()

    # DRAM staging for rotated q/k, split per head-group so group-0 attention
    # does not depend on group-1 writes
    qkrot = [[nc.dram_tensor(f"{nm}rot{g}", [L, 512], BF, kind="Internal").ap()
              for g in range(2)] for nm in ("q", "k")]

    doA = "A" in phases
    doB = "B" in phases
    doC = "C" in phases

    with tile.TileContext(nc) as tc:
        outer_cm = tc.tile_pool(name="outer", bufs=1)
        pb_cm = tc.tile_pool(name="pBqk", bufs=2, side="right")
        pbm_cm = tc.tile_pool(name="pBm", bufs=1, side="right")
        pby03_cm = tc.tile_pool(name="pBy03", bufs=1)
        pba_cm = tc.tile_pool(name="pBa", bufs=pbab)
        pbr_cm = tc.tile_pool(name="pBr", bufs=1)
        pbv_cm = tc.tile_pool(name="pBv", bufs=1)
        pbd_cm = tc.tile_pool(name="pBd", bufs=2)
        P = {}
        with outer_cm as outer, pb_cm as pb, pbm_cm as pbm, \
             pby03_cm as pby03, pba_cm as pba, pbr_cm as pbr, \
             pbv_cm as pbv, pbd_cm as pbd:
            # v for both head groups, resident SBUF: [128(kpos), LT, 512]
            vsb = [outer.tile([128, LT, 512], F16, name=f"vsb{g}",
                              tag=f"vsb{g}") for g in range(2)]
            ones128 = outer.tile([128, 128], F16, name="ones128", tag="oc")
            nc.vector.memset(ones128, 1.0)
            ebias = outer.tile([128, 1], F32, name="ebias", tag="ebias")
            nc.vector.memset(ebias, EXP_BIAS)
            trit = pbm.tile([128, 128], F16, name="tri", tag="tri")
            nc.sync.dma_start(out=trit, in_=tri)

            qts = {}
            yall = {}
            yall[0] = pby03.tile([128, 4, 2, L], F8, name="yall0", tag="yall0")
            if not doB:
                nc.vector.memset(yall[0], 0.0)

            # ---------------- phase B helpers ----------------
            def load_qk(h):
                grp, hh = h // 4, h % 4
                qt = pb.tile([128, L], BF, name="qt", tag="qt")
                kt = pb.tile([128, L], BF, name="kt", tag="kt")
                for t, src in ((qt, qkrot[0][grp]), (kt, qkrot[1][grp])):
                    nc.sync.dma_start_transpose(
                        out=t, in_=src[:, hh * 128:(hh + 1) * 128])
                return qt, kt

            def emit_B_qc(h, qt, kt, qc):
                grp, hh = h // 4, h % 4
                nkt = 4 * qc + 4
                ypsum = P["psy"].tile([128, 512], F32, name="ypsum", tag="yp")
                dacc = pbd.tile([128, 512], F16, name="dacc", tag="dacc")
                ats = {}

                def emit_score(j):
                    m = j - 4 * qc
                    off = 128 * m if m > 0 else 0
                    w = 512 - off
                    sc = P["pss"].tile([128, 512], F32, name="sc", tag="sc")
                    nc.tensor.matmul(
                        sc[:, 0:w], kt[:, j * 128:(j + 1) * 128],
                        qt[:, qc * 512 + off:(qc + 1) * 512],
                        start=True, stop=True)
                    at = pba.tile([128, 512], F16, name="at", tag="at")
                    nc.scalar.activation(
                        out=at[:, 0:w], in_=sc[:, 0:w],
                        func=mybir.ActivationFunctionType.Exp,
                        scale=ALPHA, bias=ebias)
                    if m >= 0:
                        nc.vector.tensor_mul(at[:, 0:128], at[:, 0:128], trit)
                    ats[j] = (at, off, w)

                for j in range(min(la, nkt)):
                    emit_score(j)
                prev = None
                for j in range(nkt):
                    if j + la < nkt:
                        emit_score(j + la)
                    at, off, w = ats.pop(j)
                    nc.tensor.matmul(
                        ypsum[:, off:512],
                        vsb[grp][:, j, hh * 128:(hh + 1) * 128],
                        at[:, 0:w],
                        start=(j == 0), stop=(j == nkt - 1),
                        skip_group_check=True)
                    if j == 0:
                        prev = at
                    elif j == 1:
                        if qc == 0:
                            nc.vector.tensor_copy(
                                out=dacc[:, 0:128], in_=prev[:, 0:128])
                            nc.vector.tensor_add(
                                dacc[:, 128:512], prev[:, 128:512],
                                at[:, 0:w])
                        else:
                            nc.vector.tensor_add(dacc, prev, at)
                    else:
                        nc.vector.tensor_add(
                            dacc[:, off:512], dacc[:, off:512], at[:, 0:w])
                dpsum = P["psd"].tile([128, 512], F32, name="dpsum", tag="dp")
                nc.tensor.matmul(dpsum, ones128, dacc, start=True, stop=True)
                rbs = pbr.tile([128, 512], BF, name="rbs", tag="rbs")
                with nc.allow_low_precision("softmax recip bf16"):
                    nc.vector.reciprocal(out=rbs, in_=dpsum)
                qsl = slice(qc * 512, (qc + 1) * 512)
                ya = yall[grp]
                yb = pbv.tile([128, 512], BF, name="yb", tag="yb")
                nc.vector.tensor_mul(yb, ypsum, rbs)
                nc.gpsimd.tensor_copy(out=ya[:, hh, 1, qsl], in_=yb)
                yd = pbv.tile([128, 512], BF, name="yd", tag="yd")
                nc.gpsimd.tensor_sub(yd, yb, ya[:, hh, 1, qsl])
                nc.gpsimd.tensor_scalar_mul(ya[:, hh, 0, qsl], yd, 16.0)

            # ---------------- phase A scope + interleave ----------------
            with tc.tile_pool(name="pA", bufs=1) as pa, \
                 tc.tile_pool(name="pAw", bufs=2) as paw, \
                 tc.tile_pool(name="pAt", bufs=patb) as pat, \
                 tc.tile_pool(name="pAp", bufs=16) as pap, \
                 tc.tile_pool(name="pAo", bufs=paob) as pao:
                xall = pa.tile([128, DT, 2, L], F8, name="xall", tag="xall")
                c_sb = pa.tile([128, LT, 64], BF, name="c_sb", tag="c_sb")
                s_sb = pa.tile([128, LT, 64], BF, name="s_sb", tag="s_sb")
                wch = {}

                def load_wch(c):
                    if c >= NCH or c in wch:
                        return
                    wa = paw.tile([128, DT, 512], F8, name="wchA", tag="wchA")
                    wb = paw.tile([128, DT, 2, 512], F8, name="wchB",
                                  tag="wchB")
                    wAr = wA[:, c * 512:(c + 1) * 512].rearrange(
                        "(d p) e -> p d e", p=128)
                    wBr = wB[:, c * 1024:(c + 1) * 1024].rearrange(
                        "(d p) e -> p d e", p=128)
                    wbf = wb.rearrange("p d t e -> p d (t e)")
                    for d4 in range(DT // 4):
                        sl = slice(4 * d4, 4 * d4 + 4)
                        nc.sync.dma_start(out=wa[:, sl, :], in_=wAr[:, sl, :])
                        nc.sync.dma_start(out=wbf[:, sl, :], in_=wBr[:, sl, :])
                    wch[c] = (wa, wb)

                def load_x():
                    wa = paw.tile([128, DT, 512], F8, name="wchA", tag="wchA")
                    wb = paw.tile([128, DT, 2, 512], F8, name="wchB",
                                  tag="wchB")
                    wAr = wA[:, 0:512].rearrange("(d p) e -> p d e", p=128)
                    wBr = wB[:, 0:1024].rearrange("(d p) e -> p d e", p=128)
                    wbf = wb.rearrange("p d t e -> p d (t e)")
                    xhr = xThi.rearrange("(d p) l -> p d l", p=128)
                    xlr = xTlo.rearrange("(d p) l -> p d l", p=128)
                    nc.sync.dma_start(out=xall[:, 0:2, 1, :],
                                      in_=xhr[:, 0:2, :])
                    nc.sync.dma_start(out=wa[:, 0:4, :], in_=wAr[:, 0:4, :])
                    for d2 in range(DT // 2):
                        if d2 == 0:
                            nc.sync.dma_start(out=xall[:, 0:2, 0, :],
                                              in_=xlr[:, 0:2, :])
                            continue
                        sl = slice(2 * d2, 2 * d2 + 2)
                        nc.sync.dma_start(out=xall[:, sl, 1, :],
                                          in_=xhr[:, sl, :])
                        nc.sync.dma_start(out=xall[:, sl, 0, :],
                                          in_=xlr[:, sl, :])
                        if d2 % 2 == 1:
                            sl4 = slice(2 * d2 - 2, 2 * d2 + 2)
                            if d2 > 1:
                                nc.sync.dma_start(out=wa[:, sl4, :],
                                                  in_=wAr[:, sl4, :])
                            nc.sync.dma_start(out=wbf[:, sl4, :],
                                              in_=wBr[:, sl4, :])
                            if d2 == 1:
                                for t_sb, t_dr in ((c_sb, chalf),
                                                   (s_sb, shalf)):
                                    nc.sync.dma_start(
                                        out=t_sb,
                                        in_=t_dr.rearrange(
                                            "(i p) g -> p i g", p=128))
                    wch[0] = (wa, wb)

                def emit_A_alpha(c, i):
                    wa, _ = wch[c]
                    ls = slice(i * 128, (i + 1) * 128)
                    pnat = P["psa"].tile([128, 512], F32, name="pnat",
                                         tag="pnat")
                    for d2 in range(DT // 2):
                        nc.tensor.matmul(
                            pnat,
                            xall[:, 2 * d2:2 * d2 + 2, 1, ls],
                            wa[:, 2 * d2:2 * d2 + 2, :],
                            start=(d2 == 0), stop=False, perf_mode=DR)
                    return pnat

                def emit_A_finish(c, i, pnat):
                    _, wb = wch[c]
                    if i == 8:
                        load_wch(c + 1)
                    ls = slice(i * 128, (i + 1) * 128)
                    for d in range(DT):
                        nc.tensor.matmul(
                            pnat,
                            xall[:, d, :, ls],
                            wb[:, d, :, :],
                            start=False, stop=(d == DT - 1), perf_mode=DR)
                    emit_A_rope(c, i, pnat)

                def emit_A_rope(c, i, pnat, part=None):
                    kind, grp = _chunk_kind(c)
                    ls = slice(i * 128, (i + 1) * 128)
                    if kind == "v":
                        nc.scalar.activation(
                            out=vsb[grp][:, i, :], in_=pnat,
                            func=mybir.ActivationFunctionType.Copy,
                            scale=1.0 / 1024.0)
                        return
                    # RoPE: per-head comps are permuted (evens | odds)
                    pv = pnat.rearrange("p (hh t z) -> p hh t z",
                                        hh=4, t=2, z=64)
                    x1 = pat.tile([128, 4, 64], BF, name="x1", tag="x1")
                    x2 = pat.tile([128, 4, 64], BF, name="x2", tag="x2")
                    if part is None:
                        nc.scalar.activation(
                            out=x1, in_=pv[:, :, 0, :],
                            func=mybir.ActivationFunctionType.Copy)
                        nc.scalar.activation(
                            out=x2, in_=pv[:, :, 1, :],
                            func=mybir.ActivationFunctionType.Copy)
                    else:
                        pw = part.rearrange("p (hh t z) -> p hh t z",
                                            hh=4, t=2, z=64)
                        nc.vector.tensor_add(x1, pv[:, :, 0, :],
                                             pw[:, :, 0, :])
                        nc.vector.tensor_add(x2, pv[:, :, 1, :],
                                             pw[:, :, 1, :])
                    ct = c_sb[:, i:i + 1, :].to_broadcast((128, 4, 64))
                    st = s_sb[:, i:i + 1, :].to_broadcast((128, 4, 64))
                    t1 = pat.tile([128, 4, 64], BF, name="t1", tag="t1")
                    nc.vector.tensor_mul(t1, x1, ct)
                    t2 = pat.tile([128, 4, 64], BF, name="t2", tag="t2")
                    nc.vector.tensor_mul(t2, x2, st)
                    t3 = pat.tile([128, 4, 64], BF, name="t3", tag="t3")
                    nc.vector.tensor_mul(t3, x2, ct)
                    t4 = pat.tile([128, 4, 64], BF, name="t4", tag="t4")
                    nc.vector.tensor_mul(t4, x1, st)
                    ro = pao.tile([128, 512], BF, name="ro", tag="ro")
                    rv = ro.rearrange("p (hh t z) -> p hh t z", hh=4, t=2, z=64)
                    nc.vector.tensor_sub(rv[:, :, 0, :], t1, t2)
                    nc.vector.tensor_add(rv[:, :, 1, :], t3, t4)
                    dst = qkrot[0 if kind == "q" else 1][grp]
                    nc.sync.dma_start(out=dst[ls, :], in_=ro)

                def emit_A_tile(c, i):
                    emit_A_finish(c, i, emit_A_alpha(c, i))

                def emit_A_dmajor(c, tiles):
                    # d-major across several open psum groups: every arriving
                    # x/w piece-group unlocks one alpha+2 betas per open tile
                    wa, wb = wch[c]
                    pns = [P["psa"].tile([128, 512], F32, name="pnat",
                                         tag="pnat") for _ in tiles]
                    for d2 in range(DT // 2):
                        for t, i in enumerate(tiles):
                            ls = slice(i * 128, (i + 1) * 128)
                            nc.tensor.matmul(
                                pns[t],
                                xall[:, 2 * d2:2 * d2 + 2, 1, ls],
                                wa[:, 2 * d2:2 * d2 + 2, :],
                                start=(d2 == 0), stop=False, perf_mode=DR)
                        for dd in (2 * d2, 2 * d2 + 1):
                            for t, i in enumerate(tiles):
                                ls = slice(i * 128, (i + 1) * 128)
                                nc.tensor.matmul(
                                    pns[t],
                                    xall[:, dd, :, ls],
                                    wb[:, dd, :, :],
                                    start=False, stop=(dd == DT - 1),
                                    perf_mode=DR)
                    for t, i in enumerate(tiles):
                        emit_A_rope(c, i, pns[t])

                # ---- emission: A(g0), then heads 0-3 x A(g1) ----
                if doA:
                    load_x()
                    # chunk 0 lead-in: 7 psum banks, d-major
                    with tc.tile_pool(name="psA0", bufs=8,
                                      space="PSUM") as psa0:
                        P["psa"] = psa0
                        emit_A_dmajor(0, range(8))
                        for i in range(8, LT):
                            emit_A_tile(0, i)
                        for c in (1, 2):
                            for i in range(LT):
                                emit_A_tile(c, i)
                with tc.tile_pool(name="psS", bufs=scb, space="PSUM") as pss, \
                     tc.tile_pool(name="psY", bufs=ypb, space="PSUM") as psy, \
                     tc.tile_pool(name="psD", bufs=1, space="PSUM") as psd:
                    P["pss"], P["psy"], P["psd"] = pss, psy, psd
                    with tc.tile_pool(name="psA", bufs=psab,
                                      space="PSUM") as psa:
                        P["psa"] = psa
                        g1 = [(c, i) for c in (3, 4, 5) for i in range(LT)] \
                            if doA else []
                        gi = 0
                        qts.clear()
                        slot = 0
                        for h in range(3 if doB else 0):
                            if h not in qts:
                                qts[h] = load_qk(h)
                            qt, kt = qts.pop(h)
                            for qc in range(QC):
                                emit_B_qc(h, qt, kt, qc)
                                if qc == 0 and h < 3 and doB:
                                    qts[h + 1] = load_qk(h + 1)
                                tk = takes[slot] if takes else take
                                slot += 1
                                for _ in range(tk):
                                    if gi < len(g1):
                                        emit_A_tile(*g1[gi])
                                        gi += 1
                        while gi < len(g1):
                            emit_A_tile(*g1[gi])
                            gi += 1

            # ---------------- heads 4-7 (qc-major) + phase C ----------
            with tc.tile_pool(name="pBy47", bufs=1) as pby47, \
                 tc.tile_pool(name="pCo", bufs=4) as pco, \
                 tc.tile_pool(name="psS3", bufs=scb, space="PSUM") as pss3, \
                 tc.tile_pool(name="psY3", bufs=ypb, space="PSUM") as psy3, \
                 tc.tile_pool(name="psD3", bufs=1, space="PSUM") as psd3, \
                 tc.tile_pool(name="psC", bufs=3, space="PSUM") as psc:
                P["pss"], P["psy"], P["psd"] = pss3, psy3, psd3
                qk47 = dict(qts)   # h3 was prefetched into the pb pool
                for h in range(4, NH if doB else 4):
                    grp, hh = h // 4, h % 4
                    qt = pby47.tile([128, L], BF, name=f"qt{h}", tag=f"qt{h}")
                    kt = pby47.tile([128, L], BF, name=f"kt{h}", tag=f"kt{h}")
                    for t, src in ((qt, qkrot[0][grp]), (kt, qkrot[1][grp])):
                        nc.sync.dma_start_transpose(
                            out=t, in_=src[:, hh * 128:(hh + 1) * 128])
                    qk47[h] = (qt, kt)
                yall[1] = pby47.tile([128, 4, 2, L], F8, name="yall1",
                                     tag="yall1")
                if not doB:
                    nc.vector.memset(yall[1], 0.0)
                woat = pby47.tile([128, NH, L], F8, name="woat", tag="woat")
                nc.sync.dma_start(
                    out=woat, in_=woA.rearrange("(dd p) e -> p dd e", p=128))
                wobt = pby47.tile([128, NH, 2, L], F8, name="wobt", tag="wobt")
                nc.sync.dma_start(
                    out=wobt.rearrange("p dd t e -> p dd (t e)"),
                    in_=woB.rearrange("(dd p) e -> p dd e", p=128))

                def emit_C(e, qc, final=False):
                    esl = slice(e * 128, (e + 1) * 128)
                    qsl = slice(qc * 512, (qc + 1) * 512)
                    op = psc.tile([128, 512], F32, name="op", tag="op")
                    for g in range(2):
                        for p2 in range(2):
                            dd = 4 * g + 2 * p2
                            nc.tensor.matmul(
                                op, woat[:, dd:dd + 2, esl],
                                yall[g][:, 2 * p2:2 * p2 + 2, 1, qsl],
                                start=(g == 0 and p2 == 0), stop=False,
                                perf_mode=DR)
                    for g in range(2):
                        for hh in range(4):
                            nc.tensor.matmul(
                                op, wobt[:, 4 * g + hh, :, esl],
                                yall[g][:, hh, :, qsl],
                                start=False, stop=(g == 1 and hh == 3),
                                perf_mode=DR)
                    ot = pco.tile([128, 512], BF, name="ot", tag="ot")
                    if (final and e % 2 == 0) or (not final and e % 4 == 0):
                        nc.scalar.activation(
                            out=ot, in_=op,
                            func=mybir.ActivationFunctionType.Copy,
                            scale=1.0 / 1024.0)
                    else:
                        nc.vector.tensor_scalar_mul(ot, op, 1.0 / 1024.0)
                    nc.sync.dma_start(
                        out=outT[e * 128:(e + 1) * 128,
                                 qc * 512:(qc + 1) * 512],
                        in_=ot)

                for qc in range(QC if doB else 0):
                    for h in range(3, NH):
                        emit_B_qc(h, qk47[h][0], qk47[h][1], qc)
                    if doC and qc > 0:
                        for e in range(DT):
                            emit_C(e, qc - 1)
                if doC:
                    for qc in ([3] if doB else range(QC)):
                        for e in range(DT):
                            emit_C(e, qc, final=True)
    nc.compile()
    return nc


_NC_CACHE = None


def _get_program():
    global _NC_CACHE
    if _NC_CACHE is None:
        _NC_CACHE = build_program()
    return _NC_CACHE


def _f8(a):
    return np.clip(np.asarray(a, np.float64), -240.0, 240.0).astype(F8NP)


# within each head's 128 comps: evens first, then odds
_PERM512 = np.concatenate(
    [np.concatenate([np.arange(h * 128, (h + 1) * 128, 2),
                     np.arange(h * 128 + 1, (h + 1) * 128, 2)])
     for h in range(4)])


def _host_inputs(x, w_qkv, w_o):
    inv = 1.0 / (ROPE_BASE ** (np.arange(0, HD, 2, dtype=np.float64) / HD))
    ang = np.arange(L, dtype=np.float64)[:, None] * inv[None, :]
    chalf = (np.cos(ang) / 1024.0).astype(BF16)                # [L, 64]
    shalf = (np.sin(ang) / 1024.0).astype(BF16)
    p = np.arange(128)[:, None]
    f = np.arange(128)[None, :]
    tri = (p <= f).astype(np.float16)                          # [128, 128]

    xhi_all, xlo_all = [], []
    for b in range(B):
        xb = np.asarray(x[b], np.float64)                      # [L, D]
        xh = _f8(xb)
        xl = _f8(16.0 * (xb - xh.astype(np.float64)))
        xhi_all.append(np.ascontiguousarray(xh.T))
        xlo_all.append(np.ascontiguousarray(xl.T))

    in_maps = []
    for c in range(8):
        b, g = c % 4, c // 4
        qr = w_qkv[g * DL:(g + 1) * DL]
        kr = w_qkv[D + g * DL:D + (g + 1) * DL]
        vr = w_qkv[2 * D + g * DL:2 * D + (g + 1) * DL]
        qp = qr[np.concatenate([_PERM512, 512 + _PERM512])]
        kp = kr[np.concatenate([_PERM512, 512 + _PERM512])]
        wloc = np.concatenate([qp[:512], kp[:512], vr[:512],
                               qp[512:], kp[512:], vr[512:]],
                              axis=0).astype(np.float64)       # [3DL, D]
        w64h = _f8(64.0 * wloc)
        w1024h = (w64h.astype(np.float64) * 16.0).astype(F8NP)  # exact shift
        w16l = _f8(16.0 * (64.0 * wloc - w64h.astype(np.float64)))
        wAh = np.ascontiguousarray(w1024h.T)                    # [D, 3DL]
        w64hT, w16lT = w64h.T, w16l.T                           # [D, 3DL]
        wBh = np.ascontiguousarray(np.concatenate(
            [np.stack([w64hT[:, c2 * 512:(c2 + 1) * 512],
                       w16lT[:, c2 * 512:(c2 + 1) * 512]],
                      axis=1).reshape(D, 1024)
             for c2 in range(NCH)], axis=1))                    # [D, 6DL]
        woT64 = 64.0 * w_o[:, g * DL:(g + 1) * DL].T.astype(np.float64)
        wo64h = _f8(woT64)
        woAh = np.ascontiguousarray(
            (wo64h.astype(np.float64) * 16.0).astype(F8NP))      # [DL, L]
        wo16l = _f8(16.0 * (woT64 - wo64h.astype(np.float64)))
        woBh = np.ascontiguousarray(
            np.concatenate([wo64h[:, None, :], wo16l[:, None, :]],
                           axis=1).reshape(DL, 2 * L))
        in_maps.append({
            "xThi": xhi_all[b],
            "xTlo": xlo_all[b],
            "wA": wAh,
            "wB": wBh,
            "woA": woAh,
            "woB": woBh,
            "chalf": chalf,
            "shalf": shalf,
            "tri": tri,
        })
    return in_maps


def kernel(x, w_qkv, w_o, _trace=False):
    x = np.asarray(x, dtype=np.float32)
    w_qkv = np.asarray(w_qkv, dtype=np.float32)
    w_o = np.asarray(w_o, dtype=np.float32)
    nc = _get_program()
    in_maps = _host_inputs(x, w_qkv, w_o)
    res = run_bass_kernel_spmd(nc, in_maps, core_ids=list(range(8)),
                               trace=_trace)
    kernel.last_result = res
    parts = [r["outT"].astype(np.float32) for r in res.results]
    out = np.empty((B, L, D), dtype=np.float32)
    for b in range(B):
        out[b] = (parts[b] + parts[b + 4]).T
    return out
